# revision 8
# baseline (speedup 1.0000x reference)
"""Trainium2 Bass kernel for multi-head attention (BS=2048, D=1024, H=16, d_k=64).

Returns (output [2048,1024], attn [16,2048,2048]) like the reference.

Sharding: tensor-parallel over heads -- each of the 8 cores owns 2 heads.
Each core reads the full (host-pretransposed) q/k/v plus its head-slices of
the weights, computes its 2 heads' attention + attn output, writes its slice
of `attn` and a partial output projection.  Host sums the 8 partials and adds
the bias constants (bo + Wo@bv, which factor out exactly).

Per-core dataflow (all matmuls in float32r = full PE speed):
  - qhT/khT [128(head dims),2048] = W @ x^T projections (PSUM accum over 8
    k-chunks of D_IN, bias added on ScalarE eviction).
  - vh in natural [k-row, d] layout (lhsT = vT chunks), with a ones column
    appended so attn@V also produces softmax row sums.
  - phase N (per head, 16 q-strips): S = qhT^T @ khT -> PSUM [128,2048];
    ScalarE exp(0.125*S) with fused accum_out row-sums; VectorE reciprocal +
    tensor_scalar (per-partition) normalize; DMA the finished attn strip out.
  - phase T (per head, 2 q-halves, 16 k-strips): S^T = khT^T @ qhT; exp;
    attn@V accumulates O' [65,1024] over k-strips (row 64 = row sums);
    broadcast 1/s via a K=1 ones matmul, normalize O' on eviction.
  - output projection: out_part = O^T(both heads) @ WoT, accumulated as two
    K=64 matmuls per tile, evicted + DMA'd.
"""

import os
import sys
from contextlib import ExitStack

if "/opt/trn_rl_repo" not in sys.path:
    sys.path.insert(0, "/opt/trn_rl_repo")

import numpy as np

BS = 2048
D_IN = 1024
D_OUT = 1024
H = 16
D_K = 64
N_CORES = 8
H_LOC = H // N_CORES          # 2 heads per core
HD = H_LOC * D_K              # 128 head dims per core
KCH = D_IN // 128             # 8 contraction chunks for projections
NSTRIP = BS // 128            # 16 strips of 128
SCALE = 1.0 / np.sqrt(D_K)    # 0.125

_CACHE = {}

# Filled by the last run (for test.py): bass_utils.BassKernelResults
LAST_RESULTS = None


def _build_bass():
    import concourse.bass as bass
    import concourse.tile as tile
    import concourse.mybir as mybir
    from concourse import bacc

    f32 = mybir.dt.float32
    f32r = mybir.dt.float32r
    AF = mybir.ActivationFunctionType

    nc = bacc.Bacc(None, target_bir_lowering=False)

    qT = nc.dram_tensor("qT", [D_IN, BS], f32r, kind="ExternalInput")
    kT = nc.dram_tensor("kT", [D_IN, BS], f32r, kind="ExternalInput")
    vT = nc.dram_tensor("vT", [D_IN, BS], f32r, kind="ExternalInput")
    wqT = nc.dram_tensor("wqT", [D_IN, HD], f32r, kind="ExternalInput")
    wkT = nc.dram_tensor("wkT", [D_IN, HD], f32r, kind="ExternalInput")
    wvT = nc.dram_tensor("wvT", [D_IN, HD], f32r, kind="ExternalInput")
    woT0 = nc.dram_tensor("woT0", [D_K, D_OUT], f32r, kind="ExternalInput")
    woT1 = nc.dram_tensor("woT1", [D_K, D_OUT], f32r, kind="ExternalInput")
    onesd = nc.dram_tensor("ones", [128, 128], f32r, kind="ExternalInput")
    bq = nc.dram_tensor("bq", [HD, 1], f32, kind="ExternalInput")
    bk = nc.dram_tensor("bk", [HD, 1], f32, kind="ExternalInput")

    attn_out = nc.dram_tensor("attn", [H_LOC, BS, BS], f32, kind="ExternalOutput")
    outp = nc.dram_tensor("outp", [BS, D_OUT], f32, kind="ExternalOutput")

    with tile.TileContext(nc) as tc, ExitStack() as ctx:
        consts = ctx.enter_context(tc.tile_pool(name="consts", bufs=1))
        slabs = ctx.enter_context(tc.tile_pool(name="slabs", bufs=3))
        upool = ctx.enter_context(tc.tile_pool(name="u", bufs=3))
        apool = ctx.enter_context(tc.tile_pool(name="a", bufs=3))
        spool = ctx.enter_context(tc.tile_pool(name="s", bufs=8))
        bpool = ctx.enter_context(tc.tile_pool(name="b", bufs=2))
        outpool = ctx.enter_context(tc.tile_pool(name="outsb", bufs=3))


        # ---- constants -------------------------------------------------
        w_q = consts.tile([128, KCH, HD], f32r, name="w_q", tag="w_q")
        w_k = consts.tile([128, KCH, HD], f32r, name="w_k", tag="w_k")
        w_v = consts.tile([128, KCH, HD], f32r, name="w_v", tag="w_v")
        nc.sync.dma_start(out=w_q, in_=wqT.rearrange("(ko p) m -> p ko m", p=128))
        nc.sync.dma_start(out=w_k, in_=wkT.rearrange("(ko p) m -> p ko m", p=128))
        nc.sync.dma_start(out=w_v, in_=wvT.rearrange("(ko p) m -> p ko m", p=128))
        wo0 = consts.tile([D_K, D_OUT], f32r, name="wo0", tag="wo0")
        wo1 = consts.tile([D_K, D_OUT], f32r, name="wo1", tag="wo1")
        nc.sync.dma_start(out=wo0, in_=woT0[:, :])
        nc.sync.dma_start(out=wo1, in_=woT1[:, :])
        bq_sb = consts.tile([HD, 1], f32, name="bq_sb", tag="bq_sb")
        bk_sb = consts.tile([HD, 1], f32, name="bk_sb", tag="bk_sb")
        nc.sync.dma_start(out=bq_sb, in_=bq[:, :])
        nc.sync.dma_start(out=bk_sb, in_=bk[:, :])
        # ones on all 128 partitions: lhsT for the 1/s broadcast matmul must
        # share base_partition with its rhs (which lives on partition 64).
        ones_sb = consts.tile([128, 128], f32r, name="ones_sb", tag="ones_sb")
        nc.sync.dma_start(out=ones_sb, in_=onesd[:, :])

        qhT = consts.tile([HD, BS], f32r, name="qhT", tag="qhT")   # [128, 2048] 2 heads stacked
        khT = consts.tile([HD, BS], f32r, name="khT", tag="khT")
        # natural-layout v-heads, 16 chunks of [128 rows, 64 dims + ones col]
        vh = [consts.tile([128, NSTRIP, D_K + 1], f32r, name=f"vh{h}", tag=f"vh{h}")
              for h in range(H_LOC)]
        for h in range(H_LOC):
            # ones column (index 64) of every chunk, via strided DMA
            nc.sync.dma_start(
                out=vh[h][:, :, D_K:D_K + 1],
                in_=onesd[:, 0:NSTRIP].rearrange("p (a b) -> p a b", b=1),
            )
        o_sb = [consts.tile([D_K, BS], f32r, name=f"o{h}", tag=f"o{h}")
                for h in range(H_LOC)]

        # ---- prologue: projections + vh (own PSUM scope: 4 + 4 banks) ---
        pro_ctx = ExitStack()
        pp_proj = pro_ctx.enter_context(
            tc.tile_pool(name="pp_proj", bufs=4, space="PSUM"))
        pp_vh = pro_ctx.enter_context(
            tc.tile_pool(name="pp_vh", bufs=1, space="PSUM"))

        # xhT[:, n] += w_x[:,kc,:].T @ xT[kc, n]
        def project(x_dram, w_sb, b_sb, dst):
            psums = [pp_proj.tile([128, 512], f32, name=f"pj{nb}", tag="pj")
                     for nb in range(4)]
            for kc in range(KCH):
                slab = slabs.tile([128, BS], f32r, name="slab", tag="slab")
                nc.sync.dma_start(out=slab, in_=x_dram[kc * 128:(kc + 1) * 128, :])
                for nb in range(4):
                    nc.tensor.matmul(
                        psums[nb][:, :],
                        lhsT=w_sb[:, kc, :],
                        rhs=slab[:, nb * 512:(nb + 1) * 512],
                        start=(kc == 0),
                        stop=(kc == KCH - 1),
                    )
            for nb in range(4):
                nc.scalar.activation(
                    out=dst[:, nb * 512:(nb + 1) * 512],
                    in_=psums[nb][:, :],
                    func=AF.Identity,
                    bias=b_sb[:, 0:1],
                    scale=1.0,
                )

        project(kT, w_k, bk_sb, khT)
        project(qT, w_q, bq_sb, qhT)

        # ---- vh (natural layout), 16 accumulation groups in one psum ---
        # [128, 16, 128] = 4 banks; each 2KB zero-region covers 4 chunks, so
        # start=True only on the first chunk of each region (kc==0, bc%4==0).
        psum_v = pp_vh.tile([128, NSTRIP, 128], f32, name="psum_v", tag="pv")
        for kc in range(KCH):
            slab = slabs.tile([128, BS], f32r, name="slab", tag="slab")
            nc.sync.dma_start(out=slab, in_=vT[kc * 128:(kc + 1) * 128, :])
            for bc in range(NSTRIP):
                nc.tensor.matmul(
                    psum_v[:, bc, :],
                    lhsT=slab[:, bc * 128:(bc + 1) * 128],
                    rhs=w_v[:, kc, :],
                    start=(kc == 0 and bc % 4 == 0),
                    stop=(kc == KCH - 1),
                    skip_group_check=True,
                )
        for bc in range(NSTRIP):
            for h in range(H_LOC):
                nc.vector.tensor_copy(
                    out=vh[h][:, bc, 0:D_K],
                    in_=psum_v[:, bc, h * D_K:(h + 1) * D_K],
                )

        pro_ctx.close()

        # ---- attention phases (PSUM: pn 4 + pt/pb 2 + po 2 = 8 banks) ---
        att_ctx = ExitStack()
        pp_sn = att_ctx.enter_context(
            tc.tile_pool(name="pp_sn", bufs=1, space="PSUM"))
        pp_st = att_ctx.enter_context(
            tc.tile_pool(name="pp_st", bufs=1, space="PSUM"))
        pp_op = att_ctx.enter_context(
            tc.tile_pool(name="pp_op", bufs=1, space="PSUM"))

        for h in range(H_LOC):
            hb = h * D_K   # base partition of this head inside qhT/khT

            # ---- phase N: natural-layout scores -> normalized attn out --
            for ms in range(NSTRIP):
                pn = pp_sn.tile([128, BS], f32, name="pn", tag="pn")
                for nb in range(4):
                    nc.tensor.matmul(
                        pn[:, nb * 512:(nb + 1) * 512],
                        lhsT=qhT[hb:hb + D_K, ms * 128:(ms + 1) * 128],
                        rhs=khT[hb:hb + D_K, nb * 512:(nb + 1) * 512],
                        start=True,
                        stop=True,
                    )
                u = upool.tile([128, BS], f32, name="u", tag="u")
                s = spool.tile([128, 1], f32, name="s", tag="s")
                nc.scalar.activation(
                    out=u, in_=pn, func=AF.Exp, scale=float(SCALE), accum_out=s,
                )
                r = spool.tile([128, 1], f32, name="r", tag="r")
                nc.vector.reciprocal(out=r, in_=s)
                a = apool.tile([128, BS], f32, name="a", tag="a")
                nc.vector.tensor_scalar_mul(a, u, r[:, 0:1])
                nc.sync.dma_start(
                    out=attn_out[h, ms * 128:(ms + 1) * 128, :], in_=a,
                )

            # ---- phase T: transposed scores -> attn @ V -----------------
            for half in range(2):
                q0 = half * 1024
                po = pp_op.tile([D_K + 1, 1024], f32, name="po", tag="po")
                for ks in range(NSTRIP):
                    pt = pp_st.tile([128, 1024], f32, name="pt", tag="pt")
                    for nb in range(2):
                        nc.tensor.matmul(
                            pt[:, nb * 512:(nb + 1) * 512],
                            lhsT=khT[hb:hb + D_K, ks * 128:(ks + 1) * 128],
                            rhs=qhT[hb:hb + D_K, q0 + nb * 512:q0 + (nb + 1) * 512],
                            start=True,
                            stop=True,
                        )
                    ut = upool.tile([128, 1024], f32r, name="ut", tag="u")
                    nc.scalar.activation(out=ut, in_=pt, func=AF.Exp, scale=float(SCALE))
                    for nb in range(2):
                        nc.tensor.matmul(
                            po[:, nb * 512:(nb + 1) * 512],
                            lhsT=vh[h][:, ks, :],
                            rhs=ut[:, nb * 512:(nb + 1) * 512],
                            start=(ks == 0),
                            stop=(ks == NSTRIP - 1),
                        )
                # 1/s on partition 64 (same lane as po's sum row), then
                # broadcast over partitions: B = ones[64:65,:]^T @ rrec
                rrec = spool.tile([65, 1024], f32r, name="rrec", tag="rrec")
                with nc.allow_low_precision("f32r is fp32-grade for reciprocal"):
                    nc.vector.reciprocal(out=rrec[64:65, :], in_=po[D_K:D_K + 1, :])
                pb = pp_st.tile([128, 1024], f32, name="pb", tag="pt")
                for nb in range(2):
                    nc.tensor.matmul(
                        pb[:, nb * 512:(nb + 1) * 512],
                        lhsT=ones_sb[64:65, :],
                        rhs=rrec[64:65, nb * 512:(nb + 1) * 512],
                        start=True,
                        stop=True,
                    )
                bsb = bpool.tile([128, 1024], f32, name="bsb", tag="bsb")
                nc.scalar.copy(out=bsb, in_=pb)
                nc.vector.tensor_mul(
                    o_sb[h][:, q0:q0 + 1024], po[0:D_K, :], bsb[0:D_K, :],
                )

        att_ctx.close()

        # ---- output projection: two K=64 matmuls per tile ---------------
        pp_out = ctx.enter_context(
            tc.tile_pool(name="pp_out", bufs=2, space="PSUM"))
        wo_sb = [wo0, wo1]
        for bc in range(NSTRIP):
            pout = pp_out.tile([128, 1024], f32, name="pout", tag="pout")
            for oc in range(2):
                for h in range(H_LOC):
                    nc.tensor.matmul(
                        pout[:, oc * 512:(oc + 1) * 512],
                        lhsT=o_sb[h][:, bc * 128:(bc + 1) * 128],
                        rhs=wo_sb[h][:, oc * 512:(oc + 1) * 512],
                        start=(h == 0),
                        stop=(h == H_LOC - 1),
                    )
            osb = outpool.tile([128, 1024], f32, name="osb", tag="osb")
            nc.vector.tensor_copy(out=osb, in_=pout)
            nc.sync.dma_start(out=outp[bc * 128:(bc + 1) * 128, :], in_=osb)

    nc.compile()
    return nc


def _get_nc():
    if "nc" not in _CACHE:
        _CACHE["nc"] = _build_bass()
    return _CACHE["nc"]


def _make_in_maps(q, k, v, Wq, bq, Wk, bk, Wv, Wo):
    qT = np.ascontiguousarray(q.T)
    kT = np.ascontiguousarray(k.T)
    vT = np.ascontiguousarray(v.T)
    in_maps = []
    for c in range(N_CORES):
        sl = slice(c * HD, (c + 1) * HD)
        in_maps.append({
            "qT": qT,
            "kT": kT,
            "vT": vT,
            "wqT": np.ascontiguousarray(Wq[sl, :].T),
            "wkT": np.ascontiguousarray(Wk[sl, :].T),
            "wvT": np.ascontiguousarray(Wv[sl, :].T),
            "woT0": np.ascontiguousarray(Wo[:, c * HD:c * HD + D_K].T),
            "woT1": np.ascontiguousarray(Wo[:, c * HD + D_K:(c + 1) * HD].T),
            "ones": np.ones((128, 128), dtype=np.float32),
            "bq": np.ascontiguousarray(bq[sl].reshape(HD, 1)),
            "bk": np.ascontiguousarray(bk[sl].reshape(HD, 1)),
        })
    return in_maps


def kernel(q, k, v, Wq, bq, Wk, bk, Wv, bv, Wo, bo):
    global LAST_RESULTS
    from concourse.bass_utils import run_bass_kernel_spmd

    q = np.ascontiguousarray(np.asarray(q, dtype=np.float32))
    k = np.ascontiguousarray(np.asarray(k, dtype=np.float32))
    v = np.ascontiguousarray(np.asarray(v, dtype=np.float32))
    Wq = np.asarray(Wq, dtype=np.float32)
    Wk = np.asarray(Wk, dtype=np.float32)
    Wv = np.asarray(Wv, dtype=np.float32)
    Wo = np.asarray(Wo, dtype=np.float32)
    bq = np.asarray(bq, dtype=np.float32)
    bk = np.asarray(bk, dtype=np.float32)
    bv = np.asarray(bv, dtype=np.float32)
    bo = np.asarray(bo, dtype=np.float32)

    in_maps = _make_in_maps(q, k, v, Wq, bq, Wk, bk, Wv, Wo)

    nc = _get_nc()
    res = run_bass_kernel_spmd(
        nc, in_maps, core_ids=list(range(N_CORES)),
    )
    LAST_RESULTS = res

    attn = np.concatenate([res.results[c]["attn"] for c in range(N_CORES)], axis=0)
    out = np.zeros((BS, D_OUT), dtype=np.float64)
    for c in range(N_CORES):
        out += res.results[c]["outp"]
    # bv folds through softmax (rows sum to 1) into a constant: Wo @ bv + bo
    out += (Wo.astype(np.float64) @ bv.astype(np.float64)) + bo.astype(np.float64)
    return out.astype(np.float32), attn


# revision 9
# speedup vs baseline: 1.2847x; 1.2847x over previous
"""Trainium2 Bass kernel for multi-head attention (BS=2048, D=1024, H=16, d_k=64).

Returns (output [2048,1024], attn [16,2048,2048]) like the reference.

Sharding: tensor-parallel over heads -- each of the 8 cores owns 2 heads.
Each core reads the full (host-pretransposed) q/k/v plus its head-slices of
the weights, computes its 2 heads' attention + attn output, writes its slice
of `attn` and a partial output projection.  Host sums the 8 partials and adds
the bias constants (bo + Wo@bv, which factor out exactly).

Per-core dataflow (all matmuls in float32r = full PE speed):
  - qhT/khT [128(head dims),2048] = W @ x^T projections (PSUM accum over 8
    k-chunks of D_IN, bias added on ScalarE eviction).
  - vh in natural [k-row, d] layout (lhsT = vT chunks), with a ones column
    appended so attn@V also produces softmax row sums.
  - phase N (per head, 16 q-strips): S = qhT^T @ khT -> PSUM [128,2048];
    ScalarE exp(0.125*S) with fused accum_out row-sums; VectorE reciprocal +
    tensor_scalar (per-partition) normalize; DMA the finished attn strip out.
  - phase T (per head, 2 q-halves, 16 k-strips): S^T = khT^T @ qhT; exp;
    attn@V accumulates O' [65,1024] over k-strips (row 64 = row sums);
    broadcast 1/s via a K=1 ones matmul, normalize O' on eviction.
  - output projection: out_part = O^T(both heads) @ WoT, accumulated as two
    K=64 matmuls per tile, evicted + DMA'd.
"""

import os
import sys
from contextlib import ExitStack

if "/opt/trn_rl_repo" not in sys.path:
    sys.path.insert(0, "/opt/trn_rl_repo")

import numpy as np

BS = 2048
D_IN = 1024
D_OUT = 1024
H = 16
D_K = 64
N_CORES = 8
H_LOC = H // N_CORES          # 2 heads per core
HD = H_LOC * D_K              # 128 head dims per core
KCH = D_IN // 128             # 8 contraction chunks for projections
NSTRIP = BS // 128            # 16 strips of 128
SCALE = 1.0 / np.sqrt(D_K)    # 0.125

_CACHE = {}

# Filled by the last run (for test.py): bass_utils.BassKernelResults
LAST_RESULTS = None


def _build_bass():
    import concourse.bass as bass
    import concourse.tile as tile
    import concourse.mybir as mybir
    from concourse import bacc

    f32 = mybir.dt.float32
    f32r = mybir.dt.float32r
    AF = mybir.ActivationFunctionType

    nc = bacc.Bacc(None, target_bir_lowering=False)

    qT = nc.dram_tensor("qT", [D_IN, BS], f32r, kind="ExternalInput")
    kT = nc.dram_tensor("kT", [D_IN, BS], f32r, kind="ExternalInput")
    vT = nc.dram_tensor("vT", [D_IN, BS], f32r, kind="ExternalInput")
    wqT = nc.dram_tensor("wqT", [D_IN, HD], f32r, kind="ExternalInput")
    wkT = nc.dram_tensor("wkT", [D_IN, HD], f32r, kind="ExternalInput")
    wvT = nc.dram_tensor("wvT", [D_IN, HD], f32r, kind="ExternalInput")
    woT0 = nc.dram_tensor("woT0", [D_K, D_OUT], f32r, kind="ExternalInput")
    woT1 = nc.dram_tensor("woT1", [D_K, D_OUT], f32r, kind="ExternalInput")
    onesd = nc.dram_tensor("ones", [128, 128], f32r, kind="ExternalInput")
    bq = nc.dram_tensor("bq", [HD, 1], f32, kind="ExternalInput")
    bk = nc.dram_tensor("bk", [HD, 1], f32, kind="ExternalInput")

    attn_out = nc.dram_tensor("attn", [H_LOC, BS, BS], f32, kind="ExternalOutput")
    outp = nc.dram_tensor("outp", [BS, D_OUT], f32, kind="ExternalOutput")

    with tile.TileContext(nc) as tc, ExitStack() as ctx:
        consts = ctx.enter_context(tc.tile_pool(name="consts", bufs=1))
        slabs = ctx.enter_context(tc.tile_pool(name="slabs", bufs=3))
        upool = ctx.enter_context(tc.tile_pool(name="u", bufs=3))
        apool = ctx.enter_context(tc.tile_pool(name="a", bufs=3))
        spool = ctx.enter_context(tc.tile_pool(name="s", bufs=8))
        bpool = ctx.enter_context(tc.tile_pool(name="b", bufs=2))
        outpool = ctx.enter_context(tc.tile_pool(name="outsb", bufs=3))


        # ---- constants -------------------------------------------------
        w_q = consts.tile([128, KCH, HD], f32r, name="w_q", tag="w_q")
        w_k = consts.tile([128, KCH, HD], f32r, name="w_k", tag="w_k")
        w_v = consts.tile([128, KCH, HD], f32r, name="w_v", tag="w_v")
        nc.sync.dma_start(out=w_q, in_=wqT.rearrange("(ko p) m -> p ko m", p=128))
        nc.sync.dma_start(out=w_k, in_=wkT.rearrange("(ko p) m -> p ko m", p=128))
        nc.sync.dma_start(out=w_v, in_=wvT.rearrange("(ko p) m -> p ko m", p=128))
        wo0 = consts.tile([D_K, D_OUT], f32r, name="wo0", tag="wo0")
        wo1 = consts.tile([D_K, D_OUT], f32r, name="wo1", tag="wo1")
        nc.sync.dma_start(out=wo0, in_=woT0[:, :])
        nc.sync.dma_start(out=wo1, in_=woT1[:, :])
        bq_sb = consts.tile([HD, 1], f32, name="bq_sb", tag="bq_sb")
        bk_sb = consts.tile([HD, 1], f32, name="bk_sb", tag="bk_sb")
        nc.sync.dma_start(out=bq_sb, in_=bq[:, :])
        nc.sync.dma_start(out=bk_sb, in_=bk[:, :])
        # ones on all 128 partitions: lhsT for the 1/s broadcast matmul must
        # share base_partition with its rhs (which lives on partition 64).
        ones_sb = consts.tile([128, 128], f32r, name="ones_sb", tag="ones_sb")
        nc.sync.dma_start(out=ones_sb, in_=onesd[:, :])

        qhT = consts.tile([HD, BS], f32r, name="qhT", tag="qhT")   # [128, 2048] 2 heads stacked
        khT = consts.tile([HD, BS], f32r, name="khT", tag="khT")
        # natural-layout v-heads, 16 chunks of [128 rows, 64 dims + ones col]
        vh = [consts.tile([128, NSTRIP, D_K + 1], f32r, name=f"vh{h}", tag=f"vh{h}")
              for h in range(H_LOC)]
        for h in range(H_LOC):
            # ones column (index 64) of every chunk, via strided DMA
            nc.sync.dma_start(
                out=vh[h][:, :, D_K:D_K + 1],
                in_=onesd[:, 0:NSTRIP].rearrange("p (a b) -> p a b", b=1),
            )
        o_sb = [consts.tile([D_K, BS], f32r, name=f"o{h}", tag=f"o{h}")
                for h in range(H_LOC)]

        # ---- prologue: projections + vh (own PSUM scope: 4 + 4 banks) ---
        pro_ctx = ExitStack()
        pp_proj = pro_ctx.enter_context(
            tc.tile_pool(name="pp_proj", bufs=4, space="PSUM"))
        pp_vh = pro_ctx.enter_context(
            tc.tile_pool(name="pp_vh", bufs=1, space="PSUM"))

        # xhT[:, n] += w_x[:,kc,:].T @ xT[kc, n]
        def project(x_dram, w_sb, b_sb, dst):
            psums = [pp_proj.tile([128, 512], f32, name=f"pj{nb}", tag="pj")
                     for nb in range(4)]
            for kc in range(KCH):
                slab = slabs.tile([128, BS], f32r, name="slab", tag="slab")
                nc.sync.dma_start(out=slab, in_=x_dram[kc * 128:(kc + 1) * 128, :])
                for nb in range(4):
                    nc.tensor.matmul(
                        psums[nb][:, :],
                        lhsT=w_sb[:, kc, :],
                        rhs=slab[:, nb * 512:(nb + 1) * 512],
                        start=(kc == 0),
                        stop=(kc == KCH - 1),
                    )
            for nb in range(4):
                nc.scalar.activation(
                    out=dst[:, nb * 512:(nb + 1) * 512],
                    in_=psums[nb][:, :],
                    func=AF.Identity,
                    bias=b_sb[:, 0:1],
                    scale=1.0,
                )

        project(kT, w_k, bk_sb, khT)
        project(qT, w_q, bq_sb, qhT)

        # ---- vh (natural layout), 16 accumulation groups in one psum ---
        # [128, 16, 128] = 4 banks; each 2KB zero-region covers 4 chunks, so
        # start=True only on the first chunk of each region (kc==0, bc%4==0).
        psum_v = pp_vh.tile([128, NSTRIP, 128], f32, name="psum_v", tag="pv")
        for kc in range(KCH):
            slab = slabs.tile([128, BS], f32r, name="slab", tag="slab")
            nc.sync.dma_start(out=slab, in_=vT[kc * 128:(kc + 1) * 128, :])
            for bc in range(NSTRIP):
                nc.tensor.matmul(
                    psum_v[:, bc, :],
                    lhsT=slab[:, bc * 128:(bc + 1) * 128],
                    rhs=w_v[:, kc, :],
                    start=(kc == 0 and bc % 4 == 0),
                    stop=(kc == KCH - 1),
                    skip_group_check=True,
                )
        for bc in range(NSTRIP):
            for h in range(H_LOC):
                nc.vector.tensor_copy(
                    out=vh[h][:, bc, 0:D_K],
                    in_=psum_v[:, bc, h * D_K:(h + 1) * D_K],
                )

        pro_ctx.close()

        # ---- attention phases, unit-interleaved across N and T ----------
        # PSUM: pn [128,1024]x2 = 4 banks, pt [128,1024]x1 = 2, po [65,1024]x1 = 2
        att_ctx = ExitStack()
        pp_sn = att_ctx.enter_context(
            tc.tile_pool(name="pp_sn", bufs=2, space="PSUM"))
        pp_st = att_ctx.enter_context(
            tc.tile_pool(name="pp_st", bufs=1, space="PSUM"))
        pp_op = att_ctx.enter_context(
            tc.tile_pool(name="pp_op", bufs=1, space="PSUM"))

        def finalize_o(h, qhalf, po):
            """1/s broadcast + normalized O' eviction for one q-half."""
            q0 = qhalf * 1024
            rrec = spool.tile([65, 1024], f32r, name="rrec", tag="rrec")
            with nc.allow_low_precision("f32r is fp32-grade for reciprocal"):
                nc.vector.reciprocal(out=rrec[64:65, :], in_=po[D_K:D_K + 1, :])
            pb = pp_st.tile([128, 1024], f32, name="pb", tag="pt")
            for nb in range(2):
                nc.tensor.matmul(
                    pb[:, nb * 512:(nb + 1) * 512],
                    lhsT=ones_sb[64:65, :],
                    rhs=rrec[64:65, nb * 512:(nb + 1) * 512],
                    start=True,
                    stop=True,
                )
            bsb = bpool.tile([128, 1024], f32, name="bsb", tag="bsb")
            nc.scalar.copy(out=bsb, in_=pb)
            nc.vector.tensor_mul(
                o_sb[h][:, q0:q0 + 1024], po[0:D_K, :], bsb[0:D_K, :],
            )

        for h in range(H_LOC):
            hb = h * D_K   # base partition of this head inside qhT/khT
            po_tiles = {}
            u_hold = s_hold = None
            for i in range(2 * NSTRIP):
                # ---- phase N half-strip: q-strip ms, k-half khalf --------
                ms, khalf = i // 2, i % 2
                k0 = khalf * 1024
                pn = pp_sn.tile([128, 1024], f32, name="pn", tag="pn")
                for nb in range(2):
                    nc.tensor.matmul(
                        pn[:, nb * 512:(nb + 1) * 512],
                        lhsT=qhT[hb:hb + D_K, ms * 128:(ms + 1) * 128],
                        rhs=khT[hb:hb + D_K, k0 + nb * 512:k0 + (nb + 1) * 512],
                        start=True,
                        stop=True,
                    )
                u = upool.tile([128, 1024], f32, name="u", tag="u")
                sp = spool.tile([128, 1], f32, name="sp", tag=f"sp{khalf}")
                nc.scalar.activation(
                    out=u, in_=pn, func=AF.Exp, scale=float(SCALE), accum_out=sp,
                )
                if khalf == 0:
                    u_hold, s_hold = u, sp
                else:
                    s = spool.tile([128, 1], f32, name="s", tag="s")
                    nc.vector.tensor_add(s, s_hold, sp)
                    r = spool.tile([128, 1], f32, name="r", tag="r")
                    nc.vector.reciprocal(out=r, in_=s)
                    for uu, kh in ((u_hold, 0), (u, 1)):
                        a = apool.tile([128, 1024], f32, name="a", tag="a")
                        nc.vector.tensor_scalar_mul(a, uu, r[:, 0:1])
                        nc.sync.dma_start(
                            out=attn_out[h, ms * 128:(ms + 1) * 128,
                                         kh * 1024:(kh + 1) * 1024],
                            in_=a,
                        )
                # ---- phase T strip: q-half qhalf, k-strip ks -------------
                qhalf, ks = i // NSTRIP, i % NSTRIP
                q0 = qhalf * 1024
                if ks == 0:
                    po_tiles[qhalf] = pp_op.tile(
                        [D_K + 1, 1024], f32, name="po", tag="po")
                po = po_tiles[qhalf]
                pt = pp_st.tile([128, 1024], f32, name="pt", tag="pt")
                for nb in range(2):
                    nc.tensor.matmul(
                        pt[:, nb * 512:(nb + 1) * 512],
                        lhsT=khT[hb:hb + D_K, ks * 128:(ks + 1) * 128],
                        rhs=qhT[hb:hb + D_K, q0 + nb * 512:q0 + (nb + 1) * 512],
                        start=True,
                        stop=True,
                    )
                ut = upool.tile([128, 1024], f32r, name="ut", tag="ut")
                nc.scalar.activation(out=ut, in_=pt, func=AF.Exp, scale=float(SCALE))
                for nb in range(2):
                    nc.tensor.matmul(
                        po[:, nb * 512:(nb + 1) * 512],
                        lhsT=vh[h][:, ks, :],
                        rhs=ut[:, nb * 512:(nb + 1) * 512],
                        start=(ks == 0),
                        stop=(ks == NSTRIP - 1),
                    )
                if ks == NSTRIP - 1:
                    finalize_o(h, qhalf, po)

        att_ctx.close()

        # ---- output projection: two K=64 matmuls per tile ---------------
        pp_out = ctx.enter_context(
            tc.tile_pool(name="pp_out", bufs=2, space="PSUM"))
        wo_sb = [wo0, wo1]
        for bc in range(NSTRIP):
            pout = pp_out.tile([128, 1024], f32, name="pout", tag="pout")
            for oc in range(2):
                for h in range(H_LOC):
                    nc.tensor.matmul(
                        pout[:, oc * 512:(oc + 1) * 512],
                        lhsT=o_sb[h][:, bc * 128:(bc + 1) * 128],
                        rhs=wo_sb[h][:, oc * 512:(oc + 1) * 512],
                        start=(h == 0),
                        stop=(h == H_LOC - 1),
                    )
            osb = outpool.tile([128, 1024], f32, name="osb", tag="osb")
            nc.vector.tensor_copy(out=osb, in_=pout)
            nc.sync.dma_start(out=outp[bc * 128:(bc + 1) * 128, :], in_=osb)

    nc.compile()
    return nc


def _get_nc():
    if "nc" not in _CACHE:
        _CACHE["nc"] = _build_bass()
    return _CACHE["nc"]


def _make_in_maps(q, k, v, Wq, bq, Wk, bk, Wv, Wo):
    qT = np.ascontiguousarray(q.T)
    kT = np.ascontiguousarray(k.T)
    vT = np.ascontiguousarray(v.T)
    in_maps = []
    for c in range(N_CORES):
        sl = slice(c * HD, (c + 1) * HD)
        in_maps.append({
            "qT": qT,
            "kT": kT,
            "vT": vT,
            "wqT": np.ascontiguousarray(Wq[sl, :].T),
            "wkT": np.ascontiguousarray(Wk[sl, :].T),
            "wvT": np.ascontiguousarray(Wv[sl, :].T),
            "woT0": np.ascontiguousarray(Wo[:, c * HD:c * HD + D_K].T),
            "woT1": np.ascontiguousarray(Wo[:, c * HD + D_K:(c + 1) * HD].T),
            "ones": np.ones((128, 128), dtype=np.float32),
            "bq": np.ascontiguousarray(bq[sl].reshape(HD, 1)),
            "bk": np.ascontiguousarray(bk[sl].reshape(HD, 1)),
        })
    return in_maps


def kernel(q, k, v, Wq, bq, Wk, bk, Wv, bv, Wo, bo):
    global LAST_RESULTS
    from concourse.bass_utils import run_bass_kernel_spmd

    q = np.ascontiguousarray(np.asarray(q, dtype=np.float32))
    k = np.ascontiguousarray(np.asarray(k, dtype=np.float32))
    v = np.ascontiguousarray(np.asarray(v, dtype=np.float32))
    Wq = np.asarray(Wq, dtype=np.float32)
    Wk = np.asarray(Wk, dtype=np.float32)
    Wv = np.asarray(Wv, dtype=np.float32)
    Wo = np.asarray(Wo, dtype=np.float32)
    bq = np.asarray(bq, dtype=np.float32)
    bk = np.asarray(bk, dtype=np.float32)
    bv = np.asarray(bv, dtype=np.float32)
    bo = np.asarray(bo, dtype=np.float32)

    in_maps = _make_in_maps(q, k, v, Wq, bq, Wk, bk, Wv, Wo)

    nc = _get_nc()
    res = run_bass_kernel_spmd(
        nc, in_maps, core_ids=list(range(N_CORES)),
    )
    LAST_RESULTS = res

    attn = np.concatenate([res.results[c]["attn"] for c in range(N_CORES)], axis=0)
    out = np.zeros((BS, D_OUT), dtype=np.float64)
    for c in range(N_CORES):
        out += res.results[c]["outp"]
    # bv folds through softmax (rows sum to 1) into a constant: Wo @ bv + bo
    out += (Wo.astype(np.float64) @ bv.astype(np.float64)) + bo.astype(np.float64)
    return out.astype(np.float32), attn


# revision 11
# speedup vs baseline: 1.3623x; 1.0604x over previous
"""Trainium2 Bass kernel for multi-head attention (BS=2048, D=1024, H=16, d_k=64).

Returns (output [2048,1024], attn [16,2048,2048]) like the reference.

Sharding: tensor-parallel over heads -- each of the 8 cores owns 2 heads.
Each core reads the full (host-pretransposed) q/k/v plus its head-slices of
the weights, computes its 2 heads' attention + attn output, writes its slice
of `attn` and a partial output projection.  Host sums the 8 partials and adds
the bias constants (bo + Wo@bv, which factor out exactly).

Per-core dataflow (all matmuls in float32r = full PE speed):
  - qhT/khT [128(head dims),2048] = W @ x^T projections (PSUM accum over 8
    k-chunks of D_IN, bias added on ScalarE eviction).
  - vh in natural [k-row, d] layout (lhsT = vT chunks), with a ones column
    appended so attn@V also produces softmax row sums.
  - phase N (per head, 16 q-strips): S = qhT^T @ khT -> PSUM [128,2048];
    ScalarE exp(0.125*S) with fused accum_out row-sums; VectorE reciprocal +
    tensor_scalar (per-partition) normalize; DMA the finished attn strip out.
  - phase T (per head, 2 q-halves, 16 k-strips): S^T = khT^T @ qhT; exp;
    attn@V accumulates O' [65,1024] over k-strips (row 64 = row sums);
    broadcast 1/s via a K=1 ones matmul, normalize O' on eviction.
  - output projection: out_part = O^T(both heads) @ WoT, accumulated as two
    K=64 matmuls per tile, evicted + DMA'd.
"""

import os
import sys
from contextlib import ExitStack

if "/opt/trn_rl_repo" not in sys.path:
    sys.path.insert(0, "/opt/trn_rl_repo")

import numpy as np

BS = 2048
D_IN = 1024
D_OUT = 1024
H = 16
D_K = 64
N_CORES = 8
H_LOC = H // N_CORES          # 2 heads per core
HD = H_LOC * D_K              # 128 head dims per core
KCH = D_IN // 128             # 8 contraction chunks for projections
NSTRIP = BS // 128            # 16 strips of 128
SCALE = 1.0 / np.sqrt(D_K)    # 0.125

_CACHE = {}

# Filled by the last run (for test.py): bass_utils.BassKernelResults
LAST_RESULTS = None


def _build_bass():
    import concourse.bass as bass
    import concourse.tile as tile
    import concourse.mybir as mybir
    from concourse import bacc

    f32 = mybir.dt.float32
    f32r = mybir.dt.float32r
    f16 = mybir.dt.float16
    AF = mybir.ActivationFunctionType

    nc = bacc.Bacc(None, target_bir_lowering=False)

    qT = nc.dram_tensor("qT", [D_IN, BS], f16, kind="ExternalInput")
    kT = nc.dram_tensor("kT", [D_IN, BS], f16, kind="ExternalInput")
    vT = nc.dram_tensor("vT", [D_IN, BS], f16, kind="ExternalInput")
    wqT = nc.dram_tensor("wqT", [D_IN, HD], f16, kind="ExternalInput")
    wkT = nc.dram_tensor("wkT", [D_IN, HD], f16, kind="ExternalInput")
    wvT = nc.dram_tensor("wvT", [D_IN, HD], f16, kind="ExternalInput")
    woT0 = nc.dram_tensor("woT0", [D_K, D_OUT], f16, kind="ExternalInput")
    woT1 = nc.dram_tensor("woT1", [D_K, D_OUT], f16, kind="ExternalInput")
    onesd = nc.dram_tensor("ones", [128, 128], f16, kind="ExternalInput")
    bq = nc.dram_tensor("bq", [HD, 1], f32, kind="ExternalInput")
    bk = nc.dram_tensor("bk", [HD, 1], f32, kind="ExternalInput")

    attn_out = nc.dram_tensor("attn", [H_LOC, BS, BS], f32, kind="ExternalOutput")
    outp = nc.dram_tensor("outp", [BS, D_OUT], f32, kind="ExternalOutput")

    with tile.TileContext(nc) as tc, ExitStack() as ctx:
        consts = ctx.enter_context(tc.tile_pool(name="consts", bufs=1))
        slabs = ctx.enter_context(tc.tile_pool(name="slabs", bufs=3))
        upool = ctx.enter_context(tc.tile_pool(name="u", bufs=3))
        apool = ctx.enter_context(tc.tile_pool(name="a", bufs=3))
        spool = ctx.enter_context(tc.tile_pool(name="s", bufs=8))
        bpool = ctx.enter_context(tc.tile_pool(name="b", bufs=2))
        outpool = ctx.enter_context(tc.tile_pool(name="outsb", bufs=3))


        # ---- constants -------------------------------------------------
        w_q = consts.tile([128, KCH, HD], f16, name="w_q", tag="w_q")
        w_k = consts.tile([128, KCH, HD], f16, name="w_k", tag="w_k")
        w_v = consts.tile([128, KCH, HD], f16, name="w_v", tag="w_v")
        nc.sync.dma_start(out=w_q, in_=wqT.rearrange("(ko p) m -> p ko m", p=128))
        nc.sync.dma_start(out=w_k, in_=wkT.rearrange("(ko p) m -> p ko m", p=128))
        nc.sync.dma_start(out=w_v, in_=wvT.rearrange("(ko p) m -> p ko m", p=128))
        wo0 = consts.tile([D_K, D_OUT], f16, name="wo0", tag="wo0")
        wo1 = consts.tile([D_K, D_OUT], f16, name="wo1", tag="wo1")
        nc.sync.dma_start(out=wo0, in_=woT0[:, :])
        nc.sync.dma_start(out=wo1, in_=woT1[:, :])
        bq_sb = consts.tile([HD, 1], f32, name="bq_sb", tag="bq_sb")
        bk_sb = consts.tile([HD, 1], f32, name="bk_sb", tag="bk_sb")
        nc.sync.dma_start(out=bq_sb, in_=bq[:, :])
        nc.sync.dma_start(out=bk_sb, in_=bk[:, :])
        # ones on all 128 partitions: lhsT for the 1/s broadcast matmul must
        # share base_partition with its rhs (which lives on partition 64).
        ones_sb = consts.tile([128, 128], f16, name="ones_sb", tag="ones_sb")
        nc.sync.dma_start(out=ones_sb, in_=onesd[:, :])

        qhT = consts.tile([HD, BS], f16, name="qhT", tag="qhT")   # [128, 2048] 2 heads stacked
        khT = consts.tile([HD, BS], f16, name="khT", tag="khT")
        # natural-layout v-heads, 16 chunks of [128 rows, 64 dims + ones col]
        vh = [consts.tile([128, NSTRIP, D_K + 1], f16, name=f"vh{h}", tag=f"vh{h}")
              for h in range(H_LOC)]
        for h in range(H_LOC):
            # ones column (index 64) of every chunk, via strided DMA
            nc.sync.dma_start(
                out=vh[h][:, :, D_K:D_K + 1],
                in_=onesd[:, 0:NSTRIP].rearrange("p (a b) -> p a b", b=1),
            )
        o_sb = [consts.tile([D_K, BS], f16, name=f"o{h}", tag=f"o{h}")
                for h in range(H_LOC)]

        # ---- prologue: projections + vh (own PSUM scope: 4 + 4 banks) ---
        pro_ctx = ExitStack()
        pp_proj = pro_ctx.enter_context(
            tc.tile_pool(name="pp_proj", bufs=4, space="PSUM"))
        pp_vh = pro_ctx.enter_context(
            tc.tile_pool(name="pp_vh", bufs=1, space="PSUM"))

        # xhT[:, n] += w_x[:,kc,:].T @ xT[kc, n]
        def project(x_dram, w_sb, b_sb, dst):
            psums = [pp_proj.tile([128, 512], f32, name=f"pj{nb}", tag="pj")
                     for nb in range(4)]
            for kc in range(KCH):
                slab = slabs.tile([128, BS], f16, name="slab", tag="slab")
                nc.sync.dma_start(out=slab, in_=x_dram[kc * 128:(kc + 1) * 128, :])
                for nb in range(4):
                    nc.tensor.matmul(
                        psums[nb][:, :],
                        lhsT=w_sb[:, kc, :],
                        rhs=slab[:, nb * 512:(nb + 1) * 512],
                        start=(kc == 0),
                        stop=(kc == KCH - 1),
                    )
            for nb in range(4):
                nc.scalar.activation(
                    out=dst[:, nb * 512:(nb + 1) * 512],
                    in_=psums[nb][:, :],
                    func=AF.Identity,
                    bias=b_sb[:, 0:1],
                    scale=1.0,
                )

        project(kT, w_k, bk_sb, khT)
        project(qT, w_q, bq_sb, qhT)

        # ---- vh (natural layout), 16 accumulation groups in one psum ---
        # [128, 16, 128] = 4 banks; each 2KB zero-region covers 4 chunks, so
        # start=True only on the first chunk of each region (kc==0, bc%4==0).
        psum_v = pp_vh.tile([128, NSTRIP, 128], f32, name="psum_v", tag="pv")
        for kc in range(KCH):
            slab = slabs.tile([128, BS], f16, name="slab", tag="slab")
            nc.sync.dma_start(out=slab, in_=vT[kc * 128:(kc + 1) * 128, :])
            for bc in range(NSTRIP):
                nc.tensor.matmul(
                    psum_v[:, bc, :],
                    lhsT=slab[:, bc * 128:(bc + 1) * 128],
                    rhs=w_v[:, kc, :],
                    start=(kc == 0 and bc % 4 == 0),
                    stop=(kc == KCH - 1),
                    skip_group_check=True,
                )
        for bc in range(NSTRIP):
            for h in range(H_LOC):
                nc.vector.tensor_copy(
                    out=vh[h][:, bc, 0:D_K],
                    in_=psum_v[:, bc, h * D_K:(h + 1) * D_K],
                )

        pro_ctx.close()

        # ---- attention phases: 16 units/head = 1 N-strip + 2 T-strips ---
        # PSUM: pn [128,2048]x1 = 4 banks, pt [128,1024]x1 = 2, po [65,1024]x1 = 2
        att_ctx = ExitStack()
        pp_sn = att_ctx.enter_context(
            tc.tile_pool(name="pp_sn", bufs=1, space="PSUM"))
        pp_st = att_ctx.enter_context(
            tc.tile_pool(name="pp_st", bufs=1, space="PSUM"))
        pp_op = att_ctx.enter_context(
            tc.tile_pool(name="pp_op", bufs=1, space="PSUM"))
        orawpool = ctx.enter_context(tc.tile_pool(name="oraw", bufs=2))

        def finalize_o(h, qhalf, po):
            """Copy O' out of PSUM (frees the slot), then normalize by 1/s."""
            q0 = qhalf * 1024
            o_raw = orawpool.tile([D_K + 1, 1024], f32, name="o_raw", tag="o_raw")
            nc.vector.tensor_copy(out=o_raw, in_=po)
            rrec = spool.tile([65, 1024], f16, name="rrec", tag="rrec")
            with nc.allow_low_precision("fp16 reciprocal feeding fp16 matmul"):
                nc.vector.reciprocal(out=rrec[64:65, :], in_=o_raw[64:65, :])
            pb = pp_st.tile([128, 1024], f32, name="pb", tag="pt")
            for nb in range(2):
                nc.tensor.matmul(
                    pb[:, nb * 512:(nb + 1) * 512],
                    lhsT=ones_sb[64:65, :],
                    rhs=rrec[64:65, nb * 512:(nb + 1) * 512],
                    start=True,
                    stop=True,
                )
            with nc.allow_low_precision("O output feeds fp16 out-projection"):
                nc.vector.tensor_mul(
                    o_sb[h][:, q0:q0 + 1024], o_raw[0:D_K, :], pb[0:D_K, :],
                )

        for h in range(H_LOC):
            hb = h * D_K   # base partition of this head inside qhT/khT
            po_tiles = {}
            for i in range(NSTRIP):
                # ---- phase N strip ms: full-k scores + softmax + attn out
                ms = i
                pn = pp_sn.tile([128, BS], f32, name="pn", tag="pn")
                for nb in range(4):
                    nc.tensor.matmul(
                        pn[:, nb * 512:(nb + 1) * 512],
                        lhsT=qhT[hb:hb + D_K, ms * 128:(ms + 1) * 128],
                        rhs=khT[hb:hb + D_K, nb * 512:(nb + 1) * 512],
                        start=True,
                        stop=True,
                    )
                u = upool.tile([128, BS], f32, name="u", tag="u")
                s = spool.tile([128, 1], f32, name="s", tag="s")
                nc.scalar.activation(
                    out=u, in_=pn, func=AF.Exp, scale=float(SCALE), accum_out=s,
                )
                r = spool.tile([128, 1], f32, name="r", tag="r")
                nc.vector.reciprocal(out=r, in_=s)
                a = apool.tile([128, BS], f32, name="a", tag="a")
                nc.vector.tensor_scalar_mul(a, u, r[:, 0:1])
                nc.sync.dma_start(
                    out=attn_out[h, ms * 128:(ms + 1) * 128, :], in_=a,
                )
                # ---- two T strips: S^T -> exp -> attn@V accumulate ------
                qhalf = i // 8
                q0 = qhalf * 1024
                for ks in (2 * (i % 8), 2 * (i % 8) + 1):
                    if ks == 0:
                        po_tiles[qhalf] = pp_op.tile(
                            [D_K + 1, 1024], f32, name="po", tag="po")
                    po = po_tiles[qhalf]
                    pt = pp_st.tile([128, 1024], f32, name="pt", tag="pt")
                    for nb in range(2):
                        nc.tensor.matmul(
                            pt[:, nb * 512:(nb + 1) * 512],
                            lhsT=khT[hb:hb + D_K, ks * 128:(ks + 1) * 128],
                            rhs=qhT[hb:hb + D_K, q0 + nb * 512:q0 + (nb + 1) * 512],
                            start=True,
                            stop=True,
                        )
                    ut = upool.tile([128, 1024], f16, name="ut", tag="ut")
                    nc.scalar.activation(
                        out=ut, in_=pt, func=AF.Exp, scale=float(SCALE))
                    for nb in range(2):
                        nc.tensor.matmul(
                            po[:, nb * 512:(nb + 1) * 512],
                            lhsT=vh[h][:, ks, :],
                            rhs=ut[:, nb * 512:(nb + 1) * 512],
                            start=(ks == 0),
                            stop=(ks == NSTRIP - 1),
                        )
                    if ks == NSTRIP - 1:
                        finalize_o(h, qhalf, po)

        att_ctx.close()

        # ---- output projection: two K=64 matmuls per tile ---------------
        pp_out = ctx.enter_context(
            tc.tile_pool(name="pp_out", bufs=2, space="PSUM"))
        wo_sb = [wo0, wo1]
        for bc in range(NSTRIP):
            pout = pp_out.tile([128, 1024], f32, name="pout", tag="pout")
            for oc in range(2):
                for h in range(H_LOC):
                    nc.tensor.matmul(
                        pout[:, oc * 512:(oc + 1) * 512],
                        lhsT=o_sb[h][:, bc * 128:(bc + 1) * 128],
                        rhs=wo_sb[h][:, oc * 512:(oc + 1) * 512],
                        start=(h == 0),
                        stop=(h == H_LOC - 1),
                    )
            osb = outpool.tile([128, 1024], f32, name="osb", tag="osb")
            nc.vector.tensor_copy(out=osb, in_=pout)
            nc.sync.dma_start(out=outp[bc * 128:(bc + 1) * 128, :], in_=osb)

    nc.compile()
    return nc


def _get_nc():
    if "nc" not in _CACHE:
        _CACHE["nc"] = _build_bass()
    return _CACHE["nc"]


def _make_in_maps(q, k, v, Wq, bq, Wk, bk, Wv, Wo):
    qT = np.ascontiguousarray(q.T.astype(np.float16))
    kT = np.ascontiguousarray(k.T.astype(np.float16))
    vT = np.ascontiguousarray(v.T.astype(np.float16))
    in_maps = []
    for c in range(N_CORES):
        sl = slice(c * HD, (c + 1) * HD)
        in_maps.append({
            "qT": qT,
            "kT": kT,
            "vT": vT,
            "wqT": np.ascontiguousarray(Wq[sl, :].T.astype(np.float16)),
            "wkT": np.ascontiguousarray(Wk[sl, :].T.astype(np.float16)),
            "wvT": np.ascontiguousarray(Wv[sl, :].T.astype(np.float16)),
            "woT0": np.ascontiguousarray(Wo[:, c * HD:c * HD + D_K].T.astype(np.float16)),
            "woT1": np.ascontiguousarray(Wo[:, c * HD + D_K:(c + 1) * HD].T.astype(np.float16)),
            "ones": np.ones((128, 128), dtype=np.float16),
            "bq": np.ascontiguousarray(bq[sl].reshape(HD, 1)),
            "bk": np.ascontiguousarray(bk[sl].reshape(HD, 1)),
        })
    return in_maps


def kernel(q, k, v, Wq, bq, Wk, bk, Wv, bv, Wo, bo):
    global LAST_RESULTS
    from concourse.bass_utils import run_bass_kernel_spmd

    q = np.ascontiguousarray(np.asarray(q, dtype=np.float32))
    k = np.ascontiguousarray(np.asarray(k, dtype=np.float32))
    v = np.ascontiguousarray(np.asarray(v, dtype=np.float32))
    Wq = np.asarray(Wq, dtype=np.float32)
    Wk = np.asarray(Wk, dtype=np.float32)
    Wv = np.asarray(Wv, dtype=np.float32)
    Wo = np.asarray(Wo, dtype=np.float32)
    bq = np.asarray(bq, dtype=np.float32)
    bk = np.asarray(bk, dtype=np.float32)
    bv = np.asarray(bv, dtype=np.float32)
    bo = np.asarray(bo, dtype=np.float32)

    in_maps = _make_in_maps(q, k, v, Wq, bq, Wk, bk, Wv, Wo)

    nc = _get_nc()
    res = run_bass_kernel_spmd(
        nc, in_maps, core_ids=list(range(N_CORES)),
    )
    LAST_RESULTS = res

    attn = np.concatenate([res.results[c]["attn"] for c in range(N_CORES)], axis=0)
    out = np.zeros((BS, D_OUT), dtype=np.float64)
    for c in range(N_CORES):
        out += res.results[c]["outp"]
    # bv folds through softmax (rows sum to 1) into a constant: Wo @ bv + bo
    out += (Wo.astype(np.float64) @ bv.astype(np.float64)) + bo.astype(np.float64)
    return out.astype(np.float32), attn


# revision 12
# speedup vs baseline: 1.5406x; 1.1309x over previous
"""Trainium2 Bass kernel for multi-head attention (BS=2048, D=1024, H=16, d_k=64).

Returns (output [2048,1024], attn [16,2048,2048]) like the reference.

Sharding: tensor-parallel over heads -- each of the 8 cores owns 2 heads.
Each core reads the full (host-pretransposed) q/k/v plus its head-slices of
the weights, computes its 2 heads' attention + attn output, writes its slice
of `attn` and a partial output projection.  Host sums the 8 partials and adds
the bias constants (bo + Wo@bv, which factor out exactly).

Per-core dataflow (all matmuls in float32r = full PE speed):
  - qhT/khT [128(head dims),2048] = W @ x^T projections (PSUM accum over 8
    k-chunks of D_IN, bias added on ScalarE eviction).
  - vh in natural [k-row, d] layout (lhsT = vT chunks), with a ones column
    appended so attn@V also produces softmax row sums.
  - phase N (per head, 16 q-strips): S = qhT^T @ khT -> PSUM [128,2048];
    ScalarE exp(0.125*S) with fused accum_out row-sums; VectorE reciprocal +
    tensor_scalar (per-partition) normalize; DMA the finished attn strip out.
  - phase T (per head, 2 q-halves, 16 k-strips): S^T = khT^T @ qhT; exp;
    attn@V accumulates O' [65,1024] over k-strips (row 64 = row sums);
    broadcast 1/s via a K=1 ones matmul, normalize O' on eviction.
  - output projection: out_part = O^T(both heads) @ WoT, accumulated as two
    K=64 matmuls per tile, evicted + DMA'd.
"""

import os
import sys
from contextlib import ExitStack

if "/opt/trn_rl_repo" not in sys.path:
    sys.path.insert(0, "/opt/trn_rl_repo")

import numpy as np

BS = 2048
D_IN = 1024
D_OUT = 1024
H = 16
D_K = 64
N_CORES = 8
H_LOC = H // N_CORES          # 2 heads per core
HD = H_LOC * D_K              # 128 head dims per core
KCH = D_IN // 128             # 8 contraction chunks for projections
NSTRIP = BS // 128            # 16 strips of 128
SCALE = 1.0 / np.sqrt(D_K)    # 0.125

_CACHE = {}

# Filled by the last run (for test.py): bass_utils.BassKernelResults
LAST_RESULTS = None


def _build_bass():
    import concourse.bass as bass
    import concourse.tile as tile
    import concourse.mybir as mybir
    from concourse import bacc

    f32 = mybir.dt.float32
    f32r = mybir.dt.float32r
    f16 = mybir.dt.float16
    AF = mybir.ActivationFunctionType

    nc = bacc.Bacc(None, target_bir_lowering=False)

    qT = nc.dram_tensor("qT", [D_IN, BS], f16, kind="ExternalInput")
    kT = nc.dram_tensor("kT", [D_IN, BS], f16, kind="ExternalInput")
    vT = nc.dram_tensor("vT", [D_IN, BS], f16, kind="ExternalInput")
    wqT = nc.dram_tensor("wqT", [D_IN, HD], f16, kind="ExternalInput")
    wkT = nc.dram_tensor("wkT", [D_IN, HD], f16, kind="ExternalInput")
    wvT = nc.dram_tensor("wvT", [D_IN, HD], f16, kind="ExternalInput")
    woT0 = nc.dram_tensor("woT0", [D_K, D_OUT], f16, kind="ExternalInput")
    woT1 = nc.dram_tensor("woT1", [D_K, D_OUT], f16, kind="ExternalInput")
    onesd = nc.dram_tensor("ones", [128, 128], f16, kind="ExternalInput")
    bq = nc.dram_tensor("bq", [HD, 1], f32, kind="ExternalInput")
    bk = nc.dram_tensor("bk", [HD, 1], f32, kind="ExternalInput")

    attn_out = nc.dram_tensor("attn", [H_LOC, BS, BS], f32, kind="ExternalOutput")
    outp = nc.dram_tensor("outp", [BS, D_OUT], f32, kind="ExternalOutput")

    with tile.TileContext(nc) as tc, ExitStack() as ctx:
        consts = ctx.enter_context(tc.tile_pool(name="consts", bufs=1))
        slabs = ctx.enter_context(tc.tile_pool(name="slabs", bufs=3))
        upool = ctx.enter_context(tc.tile_pool(name="u", bufs=3))
        apool = ctx.enter_context(tc.tile_pool(name="a", bufs=3))
        spool = ctx.enter_context(tc.tile_pool(name="s", bufs=8))
        bpool = ctx.enter_context(tc.tile_pool(name="b", bufs=2))
        outpool = ctx.enter_context(tc.tile_pool(name="outsb", bufs=3))


        # ---- constants -------------------------------------------------
        w_q = consts.tile([128, KCH, HD], f16, name="w_q", tag="w_q")
        w_k = consts.tile([128, KCH, HD], f16, name="w_k", tag="w_k")
        w_v = consts.tile([128, KCH, HD], f16, name="w_v", tag="w_v")
        nc.sync.dma_start(out=w_q, in_=wqT.rearrange("(ko p) m -> p ko m", p=128))
        nc.sync.dma_start(out=w_k, in_=wkT.rearrange("(ko p) m -> p ko m", p=128))
        nc.sync.dma_start(out=w_v, in_=wvT.rearrange("(ko p) m -> p ko m", p=128))
        wo0 = consts.tile([D_K, D_OUT], f16, name="wo0", tag="wo0")
        wo1 = consts.tile([D_K, D_OUT], f16, name="wo1", tag="wo1")
        nc.sync.dma_start(out=wo0, in_=woT0[:, :])
        nc.sync.dma_start(out=wo1, in_=woT1[:, :])
        bq_sb = consts.tile([HD, 1], f32, name="bq_sb", tag="bq_sb")
        bk_sb = consts.tile([HD, 1], f32, name="bk_sb", tag="bk_sb")
        nc.sync.dma_start(out=bq_sb, in_=bq[:, :])
        nc.sync.dma_start(out=bk_sb, in_=bk[:, :])
        # ones on all 128 partitions: lhsT for the 1/s broadcast matmul must
        # share base_partition with its rhs (which lives on partition 64).
        ones_sb = consts.tile([128, 128], f16, name="ones_sb", tag="ones_sb")
        nc.sync.dma_start(out=ones_sb, in_=onesd[:, :])

        qhT = consts.tile([HD, BS], f16, name="qhT", tag="qhT")   # [128, 2048] 2 heads stacked
        khT = consts.tile([HD, BS], f16, name="khT", tag="khT")
        # natural-layout v-heads, 16 chunks of [128 rows, 64 dims + ones col]
        vh = [consts.tile([128, NSTRIP, D_K + 1], f16, name=f"vh{h}", tag=f"vh{h}")
              for h in range(H_LOC)]
        for h in range(H_LOC):
            # ones column (index 64) of every chunk, via strided DMA
            nc.sync.dma_start(
                out=vh[h][:, :, D_K:D_K + 1],
                in_=onesd[:, 0:NSTRIP].rearrange("p (a b) -> p a b", b=1),
            )
        o_sb = [consts.tile([D_K, BS], f16, name=f"o{h}", tag=f"o{h}")
                for h in range(H_LOC)]

        # ---- prologue: projections + vh (own PSUM scope: 4 + 4 banks) ---
        pro_ctx = ExitStack()
        pp_proj = pro_ctx.enter_context(
            tc.tile_pool(name="pp_proj", bufs=4, space="PSUM"))
        pp_vh = pro_ctx.enter_context(
            tc.tile_pool(name="pp_vh", bufs=1, space="PSUM"))

        # xhT[:, n] += w_x[:,kc,:].T @ xT[kc, n]
        def project(x_dram, w_sb, b_sb, dst):
            psums = [pp_proj.tile([128, 512], f32, name=f"pj{nb}", tag="pj")
                     for nb in range(4)]
            for kc in range(KCH):
                slab = slabs.tile([128, BS], f16, name="slab", tag="slab")
                nc.sync.dma_start(out=slab, in_=x_dram[kc * 128:(kc + 1) * 128, :])
                for nb in range(4):
                    nc.tensor.matmul(
                        psums[nb][:, :],
                        lhsT=w_sb[:, kc, :],
                        rhs=slab[:, nb * 512:(nb + 1) * 512],
                        start=(kc == 0),
                        stop=(kc == KCH - 1),
                    )
            for nb in range(4):
                nc.scalar.activation(
                    out=dst[:, nb * 512:(nb + 1) * 512],
                    in_=psums[nb][:, :],
                    func=AF.Identity,
                    bias=b_sb[:, 0:1],
                    scale=1.0,
                )

        project(kT, w_k, bk_sb, khT)
        project(qT, w_q, bq_sb, qhT)

        # ---- vh (natural layout), 16 accumulation groups in one psum ---
        # [128, 16, 128] = 4 banks; each 2KB zero-region covers 4 chunks, so
        # start=True only on the first chunk of each region (kc==0, bc%4==0).
        psum_v = pp_vh.tile([128, NSTRIP, 128], f32, name="psum_v", tag="pv")
        for kc in range(KCH):
            slab = slabs.tile([128, BS], f16, name="slab", tag="slab")
            nc.sync.dma_start(out=slab, in_=vT[kc * 128:(kc + 1) * 128, :])
            for bc in range(NSTRIP):
                nc.tensor.matmul(
                    psum_v[:, bc, :],
                    lhsT=slab[:, bc * 128:(bc + 1) * 128],
                    rhs=w_v[:, kc, :],
                    start=(kc == 0 and bc % 4 == 0),
                    stop=(kc == KCH - 1),
                    skip_group_check=True,
                )
        for bc in range(NSTRIP):
            for h in range(H_LOC):
                nc.vector.tensor_copy(
                    out=vh[h][:, bc, 0:D_K],
                    in_=psum_v[:, bc, h * D_K:(h + 1) * D_K],
                )

        pro_ctx.close()

        # ---- attention phases ------------------------------------------
        # PSUM: pn [128,1024]x1 = 2 banks, pt [128,1024]x1 = 2, po [65,1024]x2 = 4
        att_ctx = ExitStack()
        pp_sn = att_ctx.enter_context(
            tc.tile_pool(name="pp_sn", bufs=1, space="PSUM"))
        pp_st = att_ctx.enter_context(
            tc.tile_pool(name="pp_st", bufs=1, space="PSUM"))
        pp_op = att_ctx.enter_context(
            tc.tile_pool(name="pp_op", bufs=2, space="PSUM"))
        orawpool = ctx.enter_context(tc.tile_pool(name="oraw", bufs=2))
        rrowpool = ctx.enter_context(tc.tile_pool(name="rrow", bufs=4))

        def finalize_o(h, qhalf, po, rrow):
            """Copy O' out of PSUM (frees the slot), then normalize by 1/s
            broadcast over partitions via a K=1 ones matmul."""
            q0 = qhalf * 1024
            o_raw = orawpool.tile([D_K + 1, 1024], f32, name="o_raw", tag="o_raw")
            nc.vector.tensor_copy(out=o_raw, in_=po)
            pb = pp_st.tile([128, 1024], f32, name="pb", tag="pt")
            for nb in range(2):
                nc.tensor.matmul(
                    pb[:, nb * 512:(nb + 1) * 512],
                    lhsT=ones_sb[0:1, :],
                    rhs=rrow[0:1, nb * 512:(nb + 1) * 512],
                    start=True,
                    stop=True,
                )
            with nc.allow_low_precision("O output feeds fp16 out-projection"):
                nc.vector.tensor_mul(
                    o_sb[h][:, q0:q0 + 1024], o_raw[0:D_K, :], pb[0:D_K, :],
                )

        pending = []   # deferred finalizes: (due_tick, args)
        tick = 0
        for h in range(H_LOC):
            hb = h * D_K   # base partition of this head inside qhT/khT
            po_tiles = {}
            rrows = {}
            for i in range(2 * NSTRIP):
                while pending and pending[0][0] <= tick:
                    finalize_o(*pending.pop(0)[1])
                tick += 1
                # ---- phase N half-strip: q-strip ms, k-half khalf --------
                ms, khalf = i // 2, i % 2
                k0 = khalf * 1024
                qh8 = ms // 8           # which rrow this strip contributes to
                if khalf == 0 and ms % 8 == 0:
                    rrows[qh8] = rrowpool.tile(
                        [1, 1024], f16, name="rrow", tag="rrow")
                pn = pp_sn.tile([128, 1024], f32, name="pn", tag="pn")
                for nb in range(2):
                    nc.tensor.matmul(
                        pn[:, nb * 512:(nb + 1) * 512],
                        lhsT=qhT[hb:hb + D_K, ms * 128:(ms + 1) * 128],
                        rhs=khT[hb:hb + D_K, k0 + nb * 512:k0 + (nb + 1) * 512],
                        start=True,
                        stop=True,
                    )
                u = upool.tile([128, 1024], f32, name="u", tag="u")
                sp = spool.tile([128, 1], f32, name="sp", tag=f"sp{khalf}")
                nc.scalar.activation(
                    out=u, in_=pn, func=AF.Exp, scale=float(SCALE), accum_out=sp,
                )
                if khalf == 0:
                    u_hold, s_hold = u, sp
                else:
                    s = spool.tile([128, 1], f32, name="s", tag="s")
                    nc.vector.tensor_add(s, s_hold, sp)
                    r = spool.tile([128, 1], f32, name="r", tag="r")
                    nc.vector.reciprocal(out=r, in_=s)
                    # contribute 1/s to this q-block's row vector (cast DMA)
                    nc.gpsimd.dma_start(
                        out=rrows[qh8][0:1, (ms % 8) * 128:(ms % 8 + 1) * 128],
                        in_=r[:, 0:1],
                    )
                    for uu, kh in ((u_hold, 0), (u, 1)):
                        a = apool.tile([128, 1024], f32, name="a", tag="a")
                        nc.vector.tensor_scalar_mul(a, uu, r[:, 0:1])
                        nc.sync.dma_start(
                            out=attn_out[h, ms * 128:(ms + 1) * 128,
                                         kh * 1024:(kh + 1) * 1024],
                            in_=a,
                        )
                # ---- phase T strip: q-half qhalf, k-strip ks -------------
                qhalf, ks = i // NSTRIP, i % NSTRIP
                q0 = qhalf * 1024
                if ks == 0:
                    po_tiles[qhalf] = pp_op.tile(
                        [D_K + 1, 1024], f32, name="po", tag="po")
                po = po_tiles[qhalf]
                pt = pp_st.tile([128, 1024], f32, name="pt", tag="pt")
                for nb in range(2):
                    nc.tensor.matmul(
                        pt[:, nb * 512:(nb + 1) * 512],
                        lhsT=khT[hb:hb + D_K, ks * 128:(ks + 1) * 128],
                        rhs=qhT[hb:hb + D_K, q0 + nb * 512:q0 + (nb + 1) * 512],
                        start=True,
                        stop=True,
                    )
                ut = upool.tile([128, 1024], f16, name="ut", tag="ut")
                nc.scalar.activation(
                    out=ut, in_=pt, func=AF.Exp, scale=float(SCALE))
                for nb in range(2):
                    nc.tensor.matmul(
                        po[:, nb * 512:(nb + 1) * 512],
                        lhsT=vh[h][:, ks, :],
                        rhs=ut[:, nb * 512:(nb + 1) * 512],
                        start=(ks == 0),
                        stop=(ks == NSTRIP - 1),
                    )
                if ks == NSTRIP - 1:
                    pending.append((tick + 6, (h, qhalf, po, rrows[qhalf])))
        while pending:
            finalize_o(*pending.pop(0)[1])

        att_ctx.close()

        # ---- output projection: two K=64 matmuls per tile ---------------
        pp_out = ctx.enter_context(
            tc.tile_pool(name="pp_out", bufs=2, space="PSUM"))
        wo_sb = [wo0, wo1]
        for bc in range(NSTRIP):
            pout = pp_out.tile([128, 1024], f32, name="pout", tag="pout")
            for oc in range(2):
                for h in range(H_LOC):
                    nc.tensor.matmul(
                        pout[:, oc * 512:(oc + 1) * 512],
                        lhsT=o_sb[h][:, bc * 128:(bc + 1) * 128],
                        rhs=wo_sb[h][:, oc * 512:(oc + 1) * 512],
                        start=(h == 0),
                        stop=(h == H_LOC - 1),
                    )
            osb = outpool.tile([128, 1024], f32, name="osb", tag="osb")
            nc.vector.tensor_copy(out=osb, in_=pout)
            nc.sync.dma_start(out=outp[bc * 128:(bc + 1) * 128, :], in_=osb)

    nc.compile()
    return nc


def _get_nc():
    if "nc" not in _CACHE:
        _CACHE["nc"] = _build_bass()
    return _CACHE["nc"]


def _make_in_maps(q, k, v, Wq, bq, Wk, bk, Wv, Wo):
    qT = np.ascontiguousarray(q.T.astype(np.float16))
    kT = np.ascontiguousarray(k.T.astype(np.float16))
    vT = np.ascontiguousarray(v.T.astype(np.float16))
    in_maps = []
    for c in range(N_CORES):
        sl = slice(c * HD, (c + 1) * HD)
        in_maps.append({
            "qT": qT,
            "kT": kT,
            "vT": vT,
            "wqT": np.ascontiguousarray(Wq[sl, :].T.astype(np.float16)),
            "wkT": np.ascontiguousarray(Wk[sl, :].T.astype(np.float16)),
            "wvT": np.ascontiguousarray(Wv[sl, :].T.astype(np.float16)),
            "woT0": np.ascontiguousarray(Wo[:, c * HD:c * HD + D_K].T.astype(np.float16)),
            "woT1": np.ascontiguousarray(Wo[:, c * HD + D_K:(c + 1) * HD].T.astype(np.float16)),
            "ones": np.ones((128, 128), dtype=np.float16),
            "bq": np.ascontiguousarray(bq[sl].reshape(HD, 1)),
            "bk": np.ascontiguousarray(bk[sl].reshape(HD, 1)),
        })
    return in_maps


def kernel(q, k, v, Wq, bq, Wk, bk, Wv, bv, Wo, bo):
    global LAST_RESULTS
    from concourse.bass_utils import run_bass_kernel_spmd

    q = np.ascontiguousarray(np.asarray(q, dtype=np.float32))
    k = np.ascontiguousarray(np.asarray(k, dtype=np.float32))
    v = np.ascontiguousarray(np.asarray(v, dtype=np.float32))
    Wq = np.asarray(Wq, dtype=np.float32)
    Wk = np.asarray(Wk, dtype=np.float32)
    Wv = np.asarray(Wv, dtype=np.float32)
    Wo = np.asarray(Wo, dtype=np.float32)
    bq = np.asarray(bq, dtype=np.float32)
    bk = np.asarray(bk, dtype=np.float32)
    bv = np.asarray(bv, dtype=np.float32)
    bo = np.asarray(bo, dtype=np.float32)

    in_maps = _make_in_maps(q, k, v, Wq, bq, Wk, bk, Wv, Wo)

    nc = _get_nc()
    res = run_bass_kernel_spmd(
        nc, in_maps, core_ids=list(range(N_CORES)),
    )
    LAST_RESULTS = res

    attn = np.concatenate([res.results[c]["attn"] for c in range(N_CORES)], axis=0)
    out = np.zeros((BS, D_OUT), dtype=np.float64)
    for c in range(N_CORES):
        out += res.results[c]["outp"]
    # bv folds through softmax (rows sum to 1) into a constant: Wo @ bv + bo
    out += (Wo.astype(np.float64) @ bv.astype(np.float64)) + bo.astype(np.float64)
    return out.astype(np.float32), attn


# revision 13
# speedup vs baseline: 1.8601x; 1.2074x over previous
"""Trainium2 Bass kernel for multi-head attention (BS=2048, D=1024, H=16, d_k=64).

Returns (output [2048,1024], attn [16,2048,2048]) like the reference.

Sharding: tensor-parallel over heads -- each of the 8 cores owns 2 heads.
Each core reads the full (host-pretransposed) q/k/v plus its head-slices of
the weights, computes its 2 heads' attention + attn output, writes its slice
of `attn` and a partial output projection.  Host sums the 8 partials and adds
the bias constants (bo + Wo@bv, which factor out exactly).

Per-core dataflow (all matmuls in float32r = full PE speed):
  - qhT/khT [128(head dims),2048] = W @ x^T projections (PSUM accum over 8
    k-chunks of D_IN, bias added on ScalarE eviction).
  - vh in natural [k-row, d] layout (lhsT = vT chunks), with a ones column
    appended so attn@V also produces softmax row sums.
  - phase N (per head, 16 q-strips): S = qhT^T @ khT -> PSUM [128,2048];
    ScalarE exp(0.125*S) with fused accum_out row-sums; VectorE reciprocal +
    tensor_scalar (per-partition) normalize; DMA the finished attn strip out.
  - phase T (per head, 2 q-halves, 16 k-strips): S^T = khT^T @ qhT; exp;
    attn@V accumulates O' [65,1024] over k-strips (row 64 = row sums);
    broadcast 1/s via a K=1 ones matmul, normalize O' on eviction.
  - output projection: out_part = O^T(both heads) @ WoT, accumulated as two
    K=64 matmuls per tile, evicted + DMA'd.
"""

import os
import sys
from contextlib import ExitStack

if "/opt/trn_rl_repo" not in sys.path:
    sys.path.insert(0, "/opt/trn_rl_repo")

import numpy as np

BS = 2048
D_IN = 1024
D_OUT = 1024
H = 16
D_K = 64
N_CORES = 8
H_LOC = H // N_CORES          # 2 heads per core
HD = H_LOC * D_K              # 128 head dims per core
KCH = D_IN // 128             # 8 contraction chunks for projections
NSTRIP = BS // 128            # 16 strips of 128
SCALE = 1.0 / np.sqrt(D_K)    # 0.125

_CACHE = {}

# Filled by the last run (for test.py): bass_utils.BassKernelResults
LAST_RESULTS = None


def _build_bass():
    import concourse.bass as bass
    import concourse.tile as tile
    import concourse.mybir as mybir
    from concourse import bacc

    f32 = mybir.dt.float32
    f32r = mybir.dt.float32r
    f16 = mybir.dt.float16
    AF = mybir.ActivationFunctionType

    nc = bacc.Bacc(None, target_bir_lowering=False)

    qT = nc.dram_tensor("qT", [D_IN, BS], f16, kind="ExternalInput")
    kT = nc.dram_tensor("kT", [D_IN, BS], f16, kind="ExternalInput")
    vT = nc.dram_tensor("vT", [D_IN, BS], f16, kind="ExternalInput")
    wqT = nc.dram_tensor("wqT", [D_IN, HD], f16, kind="ExternalInput")
    wkT = nc.dram_tensor("wkT", [D_IN, HD], f16, kind="ExternalInput")
    wvT = nc.dram_tensor("wvT", [D_IN, HD], f16, kind="ExternalInput")
    woT0 = nc.dram_tensor("woT0", [D_K, D_OUT], f16, kind="ExternalInput")
    woT1 = nc.dram_tensor("woT1", [D_K, D_OUT], f16, kind="ExternalInput")
    onesd = nc.dram_tensor("ones", [128, 128], f16, kind="ExternalInput")
    bq = nc.dram_tensor("bq", [HD, 1], f32, kind="ExternalInput")
    bk = nc.dram_tensor("bk", [HD, 1], f32, kind="ExternalInput")

    attn_out = nc.dram_tensor("attn", [H_LOC, BS, BS], f32, kind="ExternalOutput")
    outp = nc.dram_tensor("outp", [BS, D_OUT], f32, kind="ExternalOutput")

    QB = 512                     # q-block width for T strips / rrow quarters
    NQB = BS // QB               # 4 quarters

    with tile.TileContext(nc) as tc, ExitStack() as ctx:
        consts = ctx.enter_context(tc.tile_pool(name="consts", bufs=1))
        slabs = ctx.enter_context(tc.tile_pool(name="slabs", bufs=3))
        upool = ctx.enter_context(tc.tile_pool(name="u", bufs=4))
        utpool = ctx.enter_context(tc.tile_pool(name="ut", bufs=3))
        apool = ctx.enter_context(tc.tile_pool(name="a", bufs=4))
        spool = ctx.enter_context(tc.tile_pool(name="s", bufs=8))
        outpool = ctx.enter_context(tc.tile_pool(name="outsb", bufs=3))
        orawpool = ctx.enter_context(tc.tile_pool(name="oraw", bufs=3))
        rrowpool = ctx.enter_context(tc.tile_pool(name="rrow", bufs=4))

        # one 8-bank PSUM budget shared by everything:
        #   pn: 2 x [128,1024] (4 banks)  -- scores-N / vh-psum / outproj
        #   pt: 2 x [128,512]  (2 banks)  -- scores-T / proj accum / B bcast
        #   po: 2 x [65,512]   (2 banks)  -- attn@V accumulators
        pp_n = ctx.enter_context(tc.tile_pool(name="pp_n", bufs=2, space="PSUM"))
        pp_t = ctx.enter_context(tc.tile_pool(name="pp_t", bufs=2, space="PSUM"))
        pp_o = ctx.enter_context(tc.tile_pool(name="pp_o", bufs=2, space="PSUM"))

        # ---- constants -------------------------------------------------
        w_q = consts.tile([128, KCH, HD], f16, name="w_q", tag="w_q")
        w_k = consts.tile([128, KCH, HD], f16, name="w_k", tag="w_k")
        w_v = consts.tile([128, KCH, HD], f16, name="w_v", tag="w_v")
        nc.sync.dma_start(out=w_q, in_=wqT.rearrange("(ko p) m -> p ko m", p=128))
        nc.sync.dma_start(out=w_k, in_=wkT.rearrange("(ko p) m -> p ko m", p=128))
        nc.sync.dma_start(out=w_v, in_=wvT.rearrange("(ko p) m -> p ko m", p=128))
        wo0 = consts.tile([D_K, D_OUT], f16, name="wo0", tag="wo0")
        wo1 = consts.tile([D_K, D_OUT], f16, name="wo1", tag="wo1")
        nc.sync.dma_start(out=wo0, in_=woT0[:, :])
        nc.sync.dma_start(out=wo1, in_=woT1[:, :])
        bq_sb = consts.tile([HD, 1], f32, name="bq_sb", tag="bq_sb")
        bk_sb = consts.tile([HD, 1], f32, name="bk_sb", tag="bk_sb")
        nc.sync.dma_start(out=bq_sb, in_=bq[:, :])
        nc.sync.dma_start(out=bk_sb, in_=bk[:, :])
        ones_sb = consts.tile([128, 128], f16, name="ones_sb", tag="ones_sb")
        nc.sync.dma_start(out=ones_sb, in_=onesd[:, :])

        qhT = consts.tile([HD, BS], f16, name="qhT", tag="qhT")
        khT = consts.tile([HD, BS], f16, name="khT", tag="khT")
        vh = [consts.tile([128, NSTRIP, D_K + 1], f16, name=f"vh{h}", tag=f"vh{h}")
              for h in range(H_LOC)]
        o_sb = [consts.tile([D_K, BS], f16, name=f"o{h}", tag=f"o{h}")
                for h in range(H_LOC)]
        for h in range(H_LOC):
            nc.sync.dma_start(
                out=vh[h][:, :, D_K:D_K + 1],
                in_=onesd[:, 0:NSTRIP].rearrange("p (a b) -> p a b", b=1),
            )

        # ---- vh first (vT stream): natural-layout v heads --------------
        # two [128,8,128] psum accumulators (pn slots), 16 groups total
        psv = [pp_n.tile([128, 8, 128], f32, name=f"psv{x}", tag="pn")
               for x in range(2)]
        vslabs = []
        for nb in range(4):
            vs = slabs.tile([128, KCH, QB], f16, name="vslab", tag="slab")
            nc.sync.dma_start(
                out=vs,
                in_=vT[:, nb * QB:(nb + 1) * QB].rearrange(
                    "(ko p) n -> p ko n", p=128),
            )
            for kc in range(KCH):
                for bx in range(4):
                    bc = nb * 4 + bx
                    nc.tensor.matmul(
                        psv[bc // 8][:, bc % 8, :],
                        lhsT=vs[:, kc, bx * 128:(bx + 1) * 128],
                        rhs=w_v[:, kc, :],
                        start=(kc == 0 and bc % 4 == 0),
                        stop=(kc == KCH - 1),
                        skip_group_check=True,
                    )
            for bx in range(4):
                bc = nb * 4 + bx
                for h in range(H_LOC):
                    with nc.allow_low_precision("fp16 v-heads feed fp16 matmul"):
                        nc.vector.tensor_copy(
                            out=vh[h][:, bc, 0:D_K],
                            in_=psv[bc // 8][:, bc % 8, h * D_K:(h + 1) * D_K],
                        )

        # ---- k/q projections, q-block-major so qhT/khT land early ------
        def project(x_dram, w_sb, b_sb, dst):
            for nb in range(4):
                slab = slabs.tile([128, KCH, QB], f16, name="slab", tag="slab")
                nc.sync.dma_start(
                    out=slab,
                    in_=x_dram[:, nb * QB:(nb + 1) * QB].rearrange(
                        "(ko p) n -> p ko n", p=128),
                )
                ps = pp_t.tile([128, QB], f32, name="pj", tag="pt")
                for kc in range(KCH):
                    nc.tensor.matmul(
                        ps,
                        lhsT=w_sb[:, kc, :],
                        rhs=slab[:, kc, :],
                        start=(kc == 0),
                        stop=(kc == KCH - 1),
                    )
                nc.scalar.activation(
                    out=dst[:, nb * QB:(nb + 1) * QB],
                    in_=ps,
                    func=AF.Identity,
                    bias=b_sb[:, 0:1],
                    scale=1.0,
                )

        project(kT, w_k, bk_sb, khT)
        project(qT, w_q, bq_sb, qhT)

        # ---- attention units -------------------------------------------
        def finalize_o(h, qb, po, rrow):
            """Copy O' out of PSUM, then normalize by the 1/s row broadcast
            over partitions via a K=1 ones matmul."""
            o_raw = orawpool.tile([D_K + 1, QB], f32, name="o_raw", tag="o_raw")
            nc.vector.tensor_copy(out=o_raw, in_=po)
            pb = pp_t.tile([128, QB], f32, name="pb", tag="pt")
            nc.tensor.matmul(
                pb, lhsT=ones_sb[0:1, :], rhs=rrow[0:1, :],
                start=True, stop=True,
            )
            with nc.allow_low_precision("O output feeds fp16 out-projection"):
                nc.vector.tensor_mul(
                    o_sb[h][:, qb * QB:(qb + 1) * QB],
                    o_raw[0:D_K, :], pb[0:D_K, :],
                )

        pending = []
        tick = 0
        for h in range(H_LOC):
            hb = h * D_K
            po_tiles = {}
            rrows = {}
            u_hold = s_hold = None
            for i in range(2 * NSTRIP):
                while pending and pending[0][0] <= tick:
                    finalize_o(*pending.pop(0)[1])
                tick += 1
                # ---- phase N half-strip (q-strip ms, k-half khalf) ------
                ms, khalf = i // 2, i % 2
                k0 = khalf * 1024
                qb = i // 8     # q-quarter index, shared with the T stream
                if i % 8 == 0:
                    rrows[qb] = rrowpool.tile([1, QB], f16, name="rrow", tag="rrow")
                pn = pp_n.tile([128, 1024], f32, name="pn", tag="pn")
                for nb in range(2):
                    nc.tensor.matmul(
                        pn[:, nb * 512:(nb + 1) * 512],
                        lhsT=qhT[hb:hb + D_K, ms * 128:(ms + 1) * 128],
                        rhs=khT[hb:hb + D_K, k0 + nb * 512:k0 + (nb + 1) * 512],
                        start=True,
                        stop=True,
                    )
                u = upool.tile([128, 1024], f32, name="u", tag="u")
                sp = spool.tile([128, 1], f32, name="sp", tag=f"sp{khalf}")
                nc.scalar.activation(
                    out=u, in_=pn, func=AF.Exp, scale=float(SCALE), accum_out=sp,
                )
                if khalf == 0:
                    u_hold, s_hold = u, sp
                else:
                    s = spool.tile([128, 1], f32, name="s", tag="s")
                    nc.vector.tensor_add(s, s_hold, sp)
                    r = spool.tile([128, 1], f32, name="r", tag="r")
                    nc.vector.reciprocal(out=r, in_=s)
                    # contribute 1/s to the q-quarter row vector (cast DMA)
                    nc.gpsimd.dma_start(
                        out=rrows[ms // 4][0:1, (ms % 4) * 128:(ms % 4 + 1) * 128],
                        in_=r[:, 0:1],
                    )
                    for uu, kh in ((u_hold, 0), (u, 1)):
                        a = apool.tile([128, 1024], f32, name="a", tag="a")
                        nc.vector.tensor_scalar_mul(a, uu, r[:, 0:1])
                        nc.sync.dma_start(
                            out=attn_out[h, ms * 128:(ms + 1) * 128,
                                         kh * 1024:(kh + 1) * 1024],
                            in_=a,
                        )
                # ---- two phase-T mini-strips (q-quarter qb, k-strip ks) --
                q0 = qb * QB
                for ks in (2 * (i % 8), 2 * (i % 8) + 1):
                    if ks == 0:
                        po_tiles[qb] = pp_o.tile(
                            [D_K + 1, QB], f32, name="po", tag="po")
                    po = po_tiles[qb]
                    pt = pp_t.tile([128, QB], f32, name="pt", tag="pt")
                    nc.tensor.matmul(
                        pt,
                        lhsT=khT[hb:hb + D_K, ks * 128:(ks + 1) * 128],
                        rhs=qhT[hb:hb + D_K, q0:q0 + QB],
                        start=True,
                        stop=True,
                    )
                    ut = utpool.tile([128, QB], f16, name="ut", tag="ut")
                    nc.scalar.activation(
                        out=ut, in_=pt, func=AF.Exp, scale=float(SCALE))
                    nc.tensor.matmul(
                        po,
                        lhsT=vh[h][:, ks, :],
                        rhs=ut,
                        start=(ks == 0),
                        stop=(ks == NSTRIP - 1),
                    )
                    if ks == NSTRIP - 1:
                        pending.append((tick + 6, (h, qb, po, rrows[qb])))
        while pending:
            finalize_o(*pending.pop(0)[1])

        # ---- output projection ------------------------------------------
        wo_sb = [wo0, wo1]
        for bc in range(NSTRIP):
            pout = pp_n.tile([128, 1024], f32, name="pout", tag="pn")
            for oc in range(2):
                for h in range(H_LOC):
                    nc.tensor.matmul(
                        pout[:, oc * 512:(oc + 1) * 512],
                        lhsT=o_sb[h][:, bc * 128:(bc + 1) * 128],
                        rhs=wo_sb[h][:, oc * 512:(oc + 1) * 512],
                        start=(h == 0),
                        stop=(h == H_LOC - 1),
                    )
            osb = outpool.tile([128, 1024], f32, name="osb", tag="osb")
            nc.vector.tensor_copy(out=osb, in_=pout)
            nc.sync.dma_start(out=outp[bc * 128:(bc + 1) * 128, :], in_=osb)

    nc.compile()
    return nc


def _get_nc():
    if "nc" not in _CACHE:
        _CACHE["nc"] = _build_bass()
    return _CACHE["nc"]


def _make_in_maps(q, k, v, Wq, bq, Wk, bk, Wv, Wo):
    qT = np.ascontiguousarray(q.T.astype(np.float16))
    kT = np.ascontiguousarray(k.T.astype(np.float16))
    vT = np.ascontiguousarray(v.T.astype(np.float16))
    in_maps = []
    for c in range(N_CORES):
        sl = slice(c * HD, (c + 1) * HD)
        in_maps.append({
            "qT": qT,
            "kT": kT,
            "vT": vT,
            "wqT": np.ascontiguousarray(Wq[sl, :].T.astype(np.float16)),
            "wkT": np.ascontiguousarray(Wk[sl, :].T.astype(np.float16)),
            "wvT": np.ascontiguousarray(Wv[sl, :].T.astype(np.float16)),
            "woT0": np.ascontiguousarray(Wo[:, c * HD:c * HD + D_K].T.astype(np.float16)),
            "woT1": np.ascontiguousarray(Wo[:, c * HD + D_K:(c + 1) * HD].T.astype(np.float16)),
            "ones": np.ones((128, 128), dtype=np.float16),
            "bq": np.ascontiguousarray(bq[sl].reshape(HD, 1)),
            "bk": np.ascontiguousarray(bk[sl].reshape(HD, 1)),
        })
    return in_maps


def kernel(q, k, v, Wq, bq, Wk, bk, Wv, bv, Wo, bo):
    global LAST_RESULTS
    from concourse.bass_utils import run_bass_kernel_spmd

    q = np.ascontiguousarray(np.asarray(q, dtype=np.float32))
    k = np.ascontiguousarray(np.asarray(k, dtype=np.float32))
    v = np.ascontiguousarray(np.asarray(v, dtype=np.float32))
    Wq = np.asarray(Wq, dtype=np.float32)
    Wk = np.asarray(Wk, dtype=np.float32)
    Wv = np.asarray(Wv, dtype=np.float32)
    Wo = np.asarray(Wo, dtype=np.float32)
    bq = np.asarray(bq, dtype=np.float32)
    bk = np.asarray(bk, dtype=np.float32)
    bv = np.asarray(bv, dtype=np.float32)
    bo = np.asarray(bo, dtype=np.float32)

    in_maps = _make_in_maps(q, k, v, Wq, bq, Wk, bk, Wv, Wo)

    nc = _get_nc()
    res = run_bass_kernel_spmd(
        nc, in_maps, core_ids=list(range(N_CORES)),
    )
    LAST_RESULTS = res

    attn = np.concatenate([res.results[c]["attn"] for c in range(N_CORES)], axis=0)
    out = np.zeros((BS, D_OUT), dtype=np.float64)
    for c in range(N_CORES):
        out += res.results[c]["outp"]
    # bv folds through softmax (rows sum to 1) into a constant: Wo @ bv + bo
    out += (Wo.astype(np.float64) @ bv.astype(np.float64)) + bo.astype(np.float64)
    return out.astype(np.float32), attn


# revision 15
# speedup vs baseline: 1.9146x; 1.0292x over previous
"""Trainium2 Bass kernel for multi-head attention (BS=2048, D=1024, H=16, d_k=64).

Returns (output [2048,1024], attn [16,2048,2048]) like the reference.

Sharding: tensor-parallel over heads -- each of the 8 cores owns 2 heads.
Each core reads the full (host-pretransposed) q/k/v plus its head-slices of
the weights, computes its 2 heads' attention + attn output, writes its slice
of `attn` and a partial output projection.  Host sums the 8 partials and adds
the bias constants (bo + Wo@bv, which factor out exactly).

Per-core dataflow (all matmuls in float32r = full PE speed):
  - qhT/khT [128(head dims),2048] = W @ x^T projections (PSUM accum over 8
    k-chunks of D_IN, bias added on ScalarE eviction).
  - vh in natural [k-row, d] layout (lhsT = vT chunks), with a ones column
    appended so attn@V also produces softmax row sums.
  - phase N (per head, 16 q-strips): S = qhT^T @ khT -> PSUM [128,2048];
    ScalarE exp(0.125*S) with fused accum_out row-sums; VectorE reciprocal +
    tensor_scalar (per-partition) normalize; DMA the finished attn strip out.
  - phase T (per head, 2 q-halves, 16 k-strips): S^T = khT^T @ qhT; exp;
    attn@V accumulates O' [65,1024] over k-strips (row 64 = row sums);
    broadcast 1/s via a K=1 ones matmul, normalize O' on eviction.
  - output projection: out_part = O^T(both heads) @ WoT, accumulated as two
    K=64 matmuls per tile, evicted + DMA'd.
"""

import os
import sys
from contextlib import ExitStack

if "/opt/trn_rl_repo" not in sys.path:
    sys.path.insert(0, "/opt/trn_rl_repo")

import numpy as np

BS = 2048
D_IN = 1024
D_OUT = 1024
H = 16
D_K = 64
N_CORES = 8
H_LOC = H // N_CORES          # 2 heads per core
HD = H_LOC * D_K              # 128 head dims per core
KCH = D_IN // 128             # 8 contraction chunks for projections
NSTRIP = BS // 128            # 16 strips of 128
SCALE = 1.0 / np.sqrt(D_K)    # 0.125

_CACHE = {}

# Filled by the last run (for test.py): bass_utils.BassKernelResults
LAST_RESULTS = None


def _build_bass():
    import concourse.bass as bass
    import concourse.tile as tile
    import concourse.mybir as mybir
    from concourse import bacc

    f32 = mybir.dt.float32
    f16 = mybir.dt.float16
    AF = mybir.ActivationFunctionType

    nc = bacc.Bacc(None, target_bir_lowering=False)

    qT = nc.dram_tensor("qT", [D_IN, BS], f16, kind="ExternalInput")
    kT = nc.dram_tensor("kT", [D_IN, BS], f16, kind="ExternalInput")
    vT = nc.dram_tensor("vT", [D_IN, BS], f16, kind="ExternalInput")
    wqT = nc.dram_tensor("wqT", [D_IN, HD], f16, kind="ExternalInput")
    wkT = nc.dram_tensor("wkT", [D_IN, HD], f16, kind="ExternalInput")
    wvT = nc.dram_tensor("wvT", [D_IN, HD], f16, kind="ExternalInput")
    woT = nc.dram_tensor("woT", [HD, D_OUT], f16, kind="ExternalInput")
    onesd = nc.dram_tensor("ones", [128, 128], f16, kind="ExternalInput")
    bq = nc.dram_tensor("bq", [HD, 1], f32, kind="ExternalInput")
    bk = nc.dram_tensor("bk", [HD, 1], f32, kind="ExternalInput")

    attn_out = nc.dram_tensor("attn", [H_LOC, BS, BS], f32, kind="ExternalOutput")
    outp = nc.dram_tensor("outp", [BS, D_OUT], f32, kind="ExternalOutput")

    QB = 512                     # q-quarter width for the T stream
    NQB = BS // QB               # 4 quarters

    with tile.TileContext(nc) as tc, ExitStack() as ctx:
        consts = ctx.enter_context(tc.tile_pool(name="consts", bufs=1))
        slabs = ctx.enter_context(tc.tile_pool(name="slabs", bufs=3))
        upool = ctx.enter_context(tc.tile_pool(name="u", bufs=6))
        utpool = ctx.enter_context(tc.tile_pool(name="ut", bufs=4))
        apool = ctx.enter_context(tc.tile_pool(name="a", bufs=4))
        spool = ctx.enter_context(tc.tile_pool(name="s", bufs=8))
        outpool = ctx.enter_context(tc.tile_pool(name="outsb", bufs=3))
        orawpool = ctx.enter_context(tc.tile_pool(name="oraw", bufs=3))
        rrowpool = ctx.enter_context(tc.tile_pool(name="rrow", bufs=6))

        # 8-bank PSUM budget:
        #   pn: 2 x [128,1024]f32 (4 banks) -- N scores (one slot per head),
        #       also vh-psum [128,8,128] and outproj [128,1024]
        #   pt: 2 x [128,512]f32 (2 banks)  -- T scores / proj accum / B bcast
        #   po: 2 x [128,512]f32 (2 banks)  -- attn@V accum, both heads packed
        pp_n = ctx.enter_context(tc.tile_pool(name="pp_n", bufs=2, space="PSUM"))
        pp_t = ctx.enter_context(tc.tile_pool(name="pp_t", bufs=2, space="PSUM"))
        pp_o = ctx.enter_context(tc.tile_pool(name="pp_o", bufs=2, space="PSUM"))

        # ---- constants -------------------------------------------------
        w_q = consts.tile([128, KCH, HD], f16, name="w_q", tag="w_q")
        w_k = consts.tile([128, KCH, HD], f16, name="w_k", tag="w_k")
        w_v = consts.tile([128, KCH, HD], f16, name="w_v", tag="w_v")
        nc.sync.dma_start(out=w_q, in_=wqT.rearrange("(ko p) m -> p ko m", p=128))
        nc.sync.dma_start(out=w_k, in_=wkT.rearrange("(ko p) m -> p ko m", p=128))
        nc.sync.dma_start(out=w_v, in_=wvT.rearrange("(ko p) m -> p ko m", p=128))
        wo_sb = consts.tile([HD, D_OUT], f16, name="wo_sb", tag="wo_sb")
        nc.sync.dma_start(out=wo_sb, in_=woT[:, :])
        bq_sb = consts.tile([HD, 1], f32, name="bq_sb", tag="bq_sb")
        bk_sb = consts.tile([HD, 1], f32, name="bk_sb", tag="bk_sb")
        nc.sync.dma_start(out=bq_sb, in_=bq[:, :])
        nc.sync.dma_start(out=bk_sb, in_=bk[:, :])
        ones_sb = consts.tile([128, 128], f16, name="ones_sb", tag="ones_sb")
        nc.sync.dma_start(out=ones_sb, in_=onesd[:, :])

        qhT = consts.tile([HD, BS], f16, name="qhT", tag="qhT")
        khT = consts.tile([HD, BS], f16, name="khT", tag="khT")
        # natural-layout v heads, both packed: cols 0-63 h0, 64-127 h1
        vhb = consts.tile([128, NSTRIP, HD], f16, name="vhb", tag="vhb")
        o_sb = consts.tile([HD, BS], f16, name="o_sb", tag="o_sb")

        # ---- vh first (vT stream) --------------------------------------
        psv = [pp_n.tile([128, 8, 128], f32, name=f"psv{x}", tag="pn")
               for x in range(2)]
        for nb in range(4):
            vs = slabs.tile([128, KCH, QB], f16, name="vslab", tag="slab")
            nc.sync.dma_start(
                out=vs,
                in_=vT[:, nb * QB:(nb + 1) * QB].rearrange(
                    "(ko p) n -> p ko n", p=128),
            )
            for kc in range(KCH):
                for bx in range(4):
                    bc = nb * 4 + bx
                    nc.tensor.matmul(
                        psv[bc // 8][:, bc % 8, :],
                        lhsT=vs[:, kc, bx * 128:(bx + 1) * 128],
                        rhs=w_v[:, kc, :],
                        start=(kc == 0 and bc % 4 == 0),
                        stop=(kc == KCH - 1),
                        skip_group_check=True,
                    )
            for bx in range(4):
                bc = nb * 4 + bx
                with nc.allow_low_precision("fp16 v-heads feed fp16 matmul"):
                    nc.vector.tensor_copy(
                        out=vhb[:, bc, :],
                        in_=psv[bc // 8][:, bc % 8, :],
                    )

        # ---- k/q projections, q-block-major ----------------------------
        def project(x_dram, w_sb, b_sb, dst):
            for nb in range(4):
                slab = slabs.tile([128, KCH, QB], f16, name="slab", tag="slab")
                nc.sync.dma_start(
                    out=slab,
                    in_=x_dram[:, nb * QB:(nb + 1) * QB].rearrange(
                        "(ko p) n -> p ko n", p=128),
                )
                ps = pp_t.tile([128, QB], f32, name="pj", tag="pt")
                for kc in range(KCH):
                    nc.tensor.matmul(
                        ps,
                        lhsT=w_sb[:, kc, :],
                        rhs=slab[:, kc, :],
                        start=(kc == 0),
                        stop=(kc == KCH - 1),
                    )
                nc.scalar.activation(
                    out=dst[:, nb * QB:(nb + 1) * QB],
                    in_=ps,
                    func=AF.Identity,
                    bias=b_sb[:, 0:1],
                    scale=1.0,
                )

        project(kT, w_k, bk_sb, khT)
        project(qT, w_q, bq_sb, qhT)

        # ---- attention: 32 units, both heads interleaved ----------------
        def finalize_o(qb, po, rrow0, rrow1):
            """Copy O' (both heads) out of PSUM, normalize by per-head 1/s
            rows broadcast over each head's partition range."""
            o_raw = orawpool.tile([128, QB], f32, name="o_raw", tag="o_raw")
            nc.vector.tensor_copy(out=o_raw, in_=po)
            pb = pp_t.tile([128, QB], f32, name="pb", tag="pt")
            nc.tensor.matmul(
                pb[0:D_K, :], lhsT=ones_sb[0:1, 0:D_K], rhs=rrow0[0:1, :],
                start=True, stop=True, skip_group_check=True,
            )
            nc.tensor.matmul(
                pb[D_K:HD, :], lhsT=ones_sb[0:1, 0:D_K], rhs=rrow1[0:1, :],
                start=True, stop=True, skip_group_check=True,
            )
            with nc.allow_low_precision("O output feeds fp16 out-projection"):
                nc.vector.tensor_mul(
                    o_sb[:, qb * QB:(qb + 1) * QB], o_raw, pb,
                )

        pending = []
        tick = 0
        po_tiles = {}
        rrows = {}
        hold = {}
        for qb in range(NQB):
            for un in range(8):
                while pending and pending[0][0] <= tick:
                    finalize_o(*pending.pop(0)[1])
                tick += 1
                ms, khalf = qb * 4 + un // 2, un % 2
                k0 = khalf * 1024
                if un == 0:
                    for h in range(H_LOC):
                        rrows[(qb, h)] = rrowpool.tile(
                            [1, QB], f16, name="rrow", tag="rrow")
                # ---- N half-strips, both heads, row-group interleaved ---
                pn_t = [pp_n.tile([128, 1024], f32, name="pn", tag="pn")
                        for _ in range(H_LOC)]
                for nb in range(2):
                    for h in range(H_LOC):
                        hb = h * D_K
                        nc.tensor.matmul(
                            pn_t[h][:, nb * 512:(nb + 1) * 512],
                            lhsT=qhT[hb:hb + D_K, ms * 128:(ms + 1) * 128],
                            rhs=khT[hb:hb + D_K, k0 + nb * 512:k0 + (nb + 1) * 512],
                            start=True,
                            stop=True,
                        )
                for h in range(H_LOC):
                    u = upool.tile([128, 1024], f32, name="u", tag="u")
                    sp = spool.tile([128, 1], f32, name="sp", tag=f"sp{khalf}{h}")
                    nc.scalar.activation(
                        out=u, in_=pn_t[h], func=AF.Exp, scale=float(SCALE),
                        accum_out=sp,
                    )
                    if khalf == 0:
                        hold[h] = (u, sp)
                    else:
                        u0, s0 = hold[h]
                        s = spool.tile([128, 1], f32, name="s", tag=f"s{h}")
                        nc.vector.tensor_add(s, s0, sp)
                        r = spool.tile([128, 1], f32, name="r", tag=f"r{h}")
                        nc.vector.reciprocal(out=r, in_=s)
                        nc.gpsimd.dma_start(
                            out=rrows[(qb, h)][0:1,
                                               (ms % 4) * 128:(ms % 4 + 1) * 128],
                            in_=r[:, 0:1],
                        )
                        for uu, kh in ((u0, 0), (u, 1)):
                            a = apool.tile([128, 1024], f32, name="a", tag="a")
                            nc.vector.tensor_scalar_mul(a, uu, r[:, 0:1])
                            nc.sync.dma_start(
                                out=attn_out[h, ms * 128:(ms + 1) * 128,
                                             kh * 1024:(kh + 1) * 1024],
                                in_=a,
                            )
                # ---- two T mini-strips, heads packed --------------------
                q0 = qb * QB
                for ks in (2 * un, 2 * un + 1):
                    if ks == 0:
                        po_tiles[qb] = pp_o.tile(
                            [128, QB], f32, name="po", tag="po")
                    po = po_tiles[qb]
                    pt_t = [pp_t.tile([128, QB], f32, name="pt", tag="pt")
                            for _ in range(H_LOC)]
                    for h in range(H_LOC):
                        hb = h * D_K
                        nc.tensor.matmul(
                            pt_t[h],
                            lhsT=khT[hb:hb + D_K, ks * 128:(ks + 1) * 128],
                            rhs=qhT[hb:hb + D_K, q0:q0 + QB],
                            start=True,
                            stop=True,
                        )
                    uts = []
                    for h in range(H_LOC):
                        ut = utpool.tile([128, QB], f16, name="ut", tag="ut")
                        nc.scalar.activation(
                            out=ut, in_=pt_t[h], func=AF.Exp, scale=float(SCALE))
                        uts.append(ut)
                    for h in range(H_LOC):
                        nc.tensor.matmul(
                            po[h * D_K:(h + 1) * D_K, :],
                            lhsT=vhb[:, ks, h * D_K:(h + 1) * D_K],
                            rhs=uts[h],
                            start=(ks == 0),
                            stop=(ks == NSTRIP - 1),
                            skip_group_check=True,
                        )
                    if ks == NSTRIP - 1:
                        pending.append(
                            (tick + 6,
                             (qb, po, rrows[(qb, 0)], rrows[(qb, 1)])))
        while pending:
            finalize_o(*pending.pop(0)[1])

        # ---- output projection: full K=128 ------------------------------
        for bc in range(NSTRIP):
            pout = pp_n.tile([128, 1024], f32, name="pout", tag="pn")
            for oc in range(2):
                nc.tensor.matmul(
                    pout[:, oc * 512:(oc + 1) * 512],
                    lhsT=o_sb[:, bc * 128:(bc + 1) * 128],
                    rhs=wo_sb[:, oc * 512:(oc + 1) * 512],
                    start=True,
                    stop=True,
                )
            osb = outpool.tile([128, 1024], f32, name="osb", tag="osb")
            nc.vector.tensor_copy(out=osb, in_=pout)
            nc.sync.dma_start(out=outp[bc * 128:(bc + 1) * 128, :], in_=osb)

    nc.compile()
    return nc


def _get_nc():
    if "nc" not in _CACHE:
        _CACHE["nc"] = _build_bass()
    return _CACHE["nc"]


def _make_in_maps(q, k, v, Wq, bq, Wk, bk, Wv, Wo):
    qT = np.ascontiguousarray(q.T.astype(np.float16))
    kT = np.ascontiguousarray(k.T.astype(np.float16))
    vT = np.ascontiguousarray(v.T.astype(np.float16))
    in_maps = []
    for c in range(N_CORES):
        sl = slice(c * HD, (c + 1) * HD)
        in_maps.append({
            "qT": qT,
            "kT": kT,
            "vT": vT,
            "wqT": np.ascontiguousarray(Wq[sl, :].T.astype(np.float16)),
            "wkT": np.ascontiguousarray(Wk[sl, :].T.astype(np.float16)),
            "wvT": np.ascontiguousarray(Wv[sl, :].T.astype(np.float16)),
            "woT": np.ascontiguousarray(Wo[:, c * HD:(c + 1) * HD].T.astype(np.float16)),
            "ones": np.ones((128, 128), dtype=np.float16),
            "bq": np.ascontiguousarray(bq[sl].reshape(HD, 1)),
            "bk": np.ascontiguousarray(bk[sl].reshape(HD, 1)),
        })
    return in_maps


def kernel(q, k, v, Wq, bq, Wk, bk, Wv, bv, Wo, bo):
    global LAST_RESULTS
    from concourse.bass_utils import run_bass_kernel_spmd

    q = np.ascontiguousarray(np.asarray(q, dtype=np.float32))
    k = np.ascontiguousarray(np.asarray(k, dtype=np.float32))
    v = np.ascontiguousarray(np.asarray(v, dtype=np.float32))
    Wq = np.asarray(Wq, dtype=np.float32)
    Wk = np.asarray(Wk, dtype=np.float32)
    Wv = np.asarray(Wv, dtype=np.float32)
    Wo = np.asarray(Wo, dtype=np.float32)
    bq = np.asarray(bq, dtype=np.float32)
    bk = np.asarray(bk, dtype=np.float32)
    bv = np.asarray(bv, dtype=np.float32)
    bo = np.asarray(bo, dtype=np.float32)

    in_maps = _make_in_maps(q, k, v, Wq, bq, Wk, bk, Wv, Wo)

    nc = _get_nc()
    res = run_bass_kernel_spmd(
        nc, in_maps, core_ids=list(range(N_CORES)),
    )
    LAST_RESULTS = res

    attn = np.concatenate([res.results[c]["attn"] for c in range(N_CORES)], axis=0)
    out = np.zeros((BS, D_OUT), dtype=np.float64)
    for c in range(N_CORES):
        out += res.results[c]["outp"]
    # bv folds through softmax (rows sum to 1) into a constant: Wo @ bv + bo
    out += (Wo.astype(np.float64) @ bv.astype(np.float64)) + bo.astype(np.float64)
    return out.astype(np.float32), attn


# revision 16
# speedup vs baseline: 2.2243x; 1.1618x over previous
"""Trainium2 Bass kernel for multi-head attention (BS=2048, D=1024, H=16, d_k=64).

Returns (output [2048,1024], attn [16,2048,2048]) like the reference.

Sharding: tensor-parallel over heads -- each of the 8 cores owns 2 heads.
Each core reads the full (host-pretransposed) q/k/v plus its head-slices of
the weights, computes its 2 heads' attention + attn output, writes its slice
of `attn` and a partial output projection.  Host sums the 8 partials and adds
the bias constants (bo + Wo@bv, which factor out exactly).

Per-core dataflow (all matmuls in float32r = full PE speed):
  - qhT/khT [128(head dims),2048] = W @ x^T projections (PSUM accum over 8
    k-chunks of D_IN, bias added on ScalarE eviction).
  - vh in natural [k-row, d] layout (lhsT = vT chunks), with a ones column
    appended so attn@V also produces softmax row sums.
  - phase N (per head, 16 q-strips): S = qhT^T @ khT -> PSUM [128,2048];
    ScalarE exp(0.125*S) with fused accum_out row-sums; VectorE reciprocal +
    tensor_scalar (per-partition) normalize; DMA the finished attn strip out.
  - phase T (per head, 2 q-halves, 16 k-strips): S^T = khT^T @ qhT; exp;
    attn@V accumulates O' [65,1024] over k-strips (row 64 = row sums);
    broadcast 1/s via a K=1 ones matmul, normalize O' on eviction.
  - output projection: out_part = O^T(both heads) @ WoT, accumulated as two
    K=64 matmuls per tile, evicted + DMA'd.
"""

import os
import sys
from contextlib import ExitStack

if "/opt/trn_rl_repo" not in sys.path:
    sys.path.insert(0, "/opt/trn_rl_repo")

import numpy as np

BS = 2048
D_IN = 1024
D_OUT = 1024
H = 16
D_K = 64
N_CORES = 8
H_LOC = H // N_CORES          # 2 heads per core
HD = H_LOC * D_K              # 128 head dims per core
KCH = D_IN // 128             # 8 contraction chunks for projections
NSTRIP = BS // 128            # 16 strips of 128
SCALE = 1.0 / np.sqrt(D_K)    # 0.125

_CACHE = {}

# Filled by the last run (for test.py): bass_utils.BassKernelResults
LAST_RESULTS = None


def _build_bass():
    import concourse.bass as bass
    import concourse.tile as tile
    import concourse.mybir as mybir
    from concourse import bacc

    f32 = mybir.dt.float32
    f16 = mybir.dt.float16
    AF = mybir.ActivationFunctionType

    nc = bacc.Bacc(None, target_bir_lowering=False)

    qT = nc.dram_tensor("qT", [D_IN, BS], f16, kind="ExternalInput")
    kT = nc.dram_tensor("kT", [D_IN, BS], f16, kind="ExternalInput")
    vT = nc.dram_tensor("vT", [D_IN, BS], f16, kind="ExternalInput")
    wqT = nc.dram_tensor("wqT", [D_IN, HD], f16, kind="ExternalInput")
    wkT = nc.dram_tensor("wkT", [D_IN, HD], f16, kind="ExternalInput")
    wvT = nc.dram_tensor("wvT", [D_IN, HD], f16, kind="ExternalInput")
    woT = nc.dram_tensor("woT", [HD, D_OUT], f16, kind="ExternalInput")
    onesd = nc.dram_tensor("ones", [128, 128], f16, kind="ExternalInput")
    bq = nc.dram_tensor("bq", [HD, 1], f32, kind="ExternalInput")
    bk = nc.dram_tensor("bk", [HD, 1], f32, kind="ExternalInput")

    attn_out = nc.dram_tensor("attn", [H_LOC, BS, BS], f32, kind="ExternalOutput")
    outp = nc.dram_tensor("outp", [BS, D_OUT], f32, kind="ExternalOutput")

    QB = 512                     # q-quarter width for the T stream
    NQB = BS // QB               # 4 quarters

    with tile.TileContext(nc) as tc, ExitStack() as ctx:
        consts = ctx.enter_context(tc.tile_pool(name="consts", bufs=1))
        slabs = ctx.enter_context(tc.tile_pool(name="slabs", bufs=3))
        upool = ctx.enter_context(tc.tile_pool(name="u", bufs=6))
        utpool = ctx.enter_context(tc.tile_pool(name="ut", bufs=4))
        apool = ctx.enter_context(tc.tile_pool(name="a", bufs=4))
        spool = ctx.enter_context(tc.tile_pool(name="s", bufs=8))
        outpool = ctx.enter_context(tc.tile_pool(name="outsb", bufs=3))
        orawpool = ctx.enter_context(tc.tile_pool(name="oraw", bufs=3))
        rrowpool = ctx.enter_context(tc.tile_pool(name="rrow", bufs=6))

        # 8-bank PSUM budget:
        #   pn: 2 x [128,1024]f32 (4 banks) -- N scores (one slot per head),
        #       also vh-psum [128,8,128] and outproj [128,1024]
        #   pt: 2 x [128,512]f32 (2 banks)  -- T scores / proj accum / B bcast
        #   po: 2 x [128,512]f32 (2 banks)  -- attn@V accum, both heads packed
        pp_n = ctx.enter_context(tc.tile_pool(name="pp_n", bufs=2, space="PSUM"))
        pp_t = ctx.enter_context(tc.tile_pool(name="pp_t", bufs=2, space="PSUM"))
        pp_o = ctx.enter_context(tc.tile_pool(name="pp_o", bufs=2, space="PSUM"))

        # ---- constants -------------------------------------------------
        w_q = consts.tile([128, KCH, HD], f16, name="w_q", tag="w_q")
        w_k = consts.tile([128, KCH, HD], f16, name="w_k", tag="w_k")
        w_v = consts.tile([128, KCH, HD], f16, name="w_v", tag="w_v")
        nc.sync.dma_start(out=w_q, in_=wqT.rearrange("(ko p) m -> p ko m", p=128))
        nc.sync.dma_start(out=w_k, in_=wkT.rearrange("(ko p) m -> p ko m", p=128))
        nc.sync.dma_start(out=w_v, in_=wvT.rearrange("(ko p) m -> p ko m", p=128))
        wo_sb = consts.tile([HD, D_OUT], f16, name="wo_sb", tag="wo_sb")
        nc.sync.dma_start(out=wo_sb, in_=woT[:, :])
        bq_sb = consts.tile([HD, 1], f32, name="bq_sb", tag="bq_sb")
        bk_sb = consts.tile([HD, 1], f32, name="bk_sb", tag="bk_sb")
        nc.sync.dma_start(out=bq_sb, in_=bq[:, :])
        nc.sync.dma_start(out=bk_sb, in_=bk[:, :])
        ones_sb = consts.tile([128, 128], f16, name="ones_sb", tag="ones_sb")
        nc.sync.dma_start(out=ones_sb, in_=onesd[:, :])

        # per-head K-padded projections: other head's partitions are zero so
        # every score matmul runs with a full K=128 contraction (full-array
        # activity keeps the PE clock unthrottled; zeros contribute nothing)
        qhT_z = [consts.tile([128, BS], f16, name=f"qhT_z{h}", tag=f"qhT_z{h}")
                 for h in range(H_LOC)]
        khT_z = [consts.tile([128, BS], f16, name=f"khT_z{h}", tag=f"khT_z{h}")
                 for h in range(H_LOC)]
        nc.vector.memset(qhT_z[0][D_K:128, :], 0.0)
        nc.vector.memset(qhT_z[1][0:D_K, :], 0.0)
        nc.vector.memset(khT_z[0][D_K:128, :], 0.0)
        nc.vector.memset(khT_z[1][0:D_K, :], 0.0)
        # natural-layout v heads, both packed: cols 0-63 h0, 64-127 h1
        vhb = consts.tile([128, NSTRIP, HD], f16, name="vhb", tag="vhb")
        o_sb = consts.tile([HD, BS], f16, name="o_sb", tag="o_sb")

        # ---- vh first (vT stream) --------------------------------------
        psv = [pp_n.tile([128, 8, 128], f32, name=f"psv{x}", tag="pn")
               for x in range(2)]
        for nb in range(4):
            vs = slabs.tile([128, KCH, QB], f16, name="vslab", tag="slab")
            nc.sync.dma_start(
                out=vs,
                in_=vT[:, nb * QB:(nb + 1) * QB].rearrange(
                    "(ko p) n -> p ko n", p=128),
            )
            for kc in range(KCH):
                for bx in range(4):
                    bc = nb * 4 + bx
                    nc.tensor.matmul(
                        psv[bc // 8][:, bc % 8, :],
                        lhsT=vs[:, kc, bx * 128:(bx + 1) * 128],
                        rhs=w_v[:, kc, :],
                        start=(kc == 0 and bc % 4 == 0),
                        stop=(kc == KCH - 1),
                        skip_group_check=True,
                    )
            for bx in range(4):
                bc = nb * 4 + bx
                with nc.allow_low_precision("fp16 v-heads feed fp16 matmul"):
                    nc.vector.tensor_copy(
                        out=vhb[:, bc, :],
                        in_=psv[bc // 8][:, bc % 8, :],
                    )

        # ---- k/q projections, q-block-major ----------------------------
        def project(x_dram, w_sb, b_sb, dst):
            for nb in range(4):
                slab = slabs.tile([128, KCH, QB], f16, name="slab", tag="slab")
                nc.sync.dma_start(
                    out=slab,
                    in_=x_dram[:, nb * QB:(nb + 1) * QB].rearrange(
                        "(ko p) n -> p ko n", p=128),
                )
                ps = pp_t.tile([128, QB], f32, name="pj", tag="pt")
                for kc in range(KCH):
                    nc.tensor.matmul(
                        ps,
                        lhsT=w_sb[:, kc, :],
                        rhs=slab[:, kc, :],
                        start=(kc == 0),
                        stop=(kc == KCH - 1),
                    )
                nc.scalar.activation(
                    out=dst[0][0:D_K, nb * QB:(nb + 1) * QB],
                    in_=ps[0:D_K, :],
                    func=AF.Identity,
                    bias=b_sb[0:D_K, 0:1],
                    scale=1.0,
                )
                nc.scalar.activation(
                    out=dst[1][D_K:128, nb * QB:(nb + 1) * QB],
                    in_=ps[D_K:128, :],
                    func=AF.Identity,
                    bias=b_sb[D_K:128, 0:1],
                    scale=1.0,
                )

        project(kT, w_k, bk_sb, khT_z)
        project(qT, w_q, bq_sb, qhT_z)

        # ---- attention: 32 units, both heads interleaved ----------------
        def finalize_o(qb, po, rrow0, rrow1):
            """Copy O' (both heads) out of PSUM, normalize by per-head 1/s
            rows broadcast over each head's partition range."""
            o_raw = orawpool.tile([128, QB], f32, name="o_raw", tag="o_raw")
            nc.vector.tensor_copy(out=o_raw, in_=po)
            pb = pp_t.tile([128, QB], f32, name="pb", tag="pt")
            nc.tensor.matmul(
                pb[0:D_K, :], lhsT=ones_sb[0:1, 0:D_K], rhs=rrow0[0:1, :],
                start=True, stop=True, skip_group_check=True,
            )
            nc.tensor.matmul(
                pb[D_K:HD, :], lhsT=ones_sb[0:1, 0:D_K], rhs=rrow1[0:1, :],
                start=True, stop=True, skip_group_check=True,
            )
            with nc.allow_low_precision("O output feeds fp16 out-projection"):
                nc.vector.tensor_mul(
                    o_sb[:, qb * QB:(qb + 1) * QB], o_raw, pb,
                )

        pending = []
        tick = 0
        po_tiles = {}
        rrows = {}
        hold = {}
        for qb in range(NQB):
            for un in range(8):
                while pending and pending[0][0] <= tick:
                    finalize_o(*pending.pop(0)[1])
                tick += 1
                ms, khalf = qb * 4 + un // 2, un % 2
                k0 = khalf * 1024
                if un == 0:
                    for h in range(H_LOC):
                        rrows[(qb, h)] = rrowpool.tile(
                            [1, QB], f16, name="rrow", tag="rrow")
                # ---- N half-strips, both heads, row-group interleaved ---
                pn_t = [pp_n.tile([128, 1024], f32, name="pn", tag="pn")
                        for _ in range(H_LOC)]
                for nb in range(2):
                    for h in range(H_LOC):
                        nc.tensor.matmul(
                            pn_t[h][:, nb * 512:(nb + 1) * 512],
                            lhsT=qhT_z[h][:, ms * 128:(ms + 1) * 128],
                            rhs=khT_z[h][:, k0 + nb * 512:k0 + (nb + 1) * 512],
                            start=True,
                            stop=True,
                        )
                for h in range(H_LOC):
                    u = upool.tile([128, 1024], f32, name="u", tag="u")
                    sp = spool.tile([128, 1], f32, name="sp", tag=f"sp{khalf}{h}")
                    nc.scalar.activation(
                        out=u, in_=pn_t[h], func=AF.Exp, scale=float(SCALE),
                        accum_out=sp,
                    )
                    if khalf == 0:
                        hold[h] = (u, sp)
                    else:
                        u0, s0 = hold[h]
                        s = spool.tile([128, 1], f32, name="s", tag=f"s{h}")
                        nc.vector.tensor_add(s, s0, sp)
                        r = spool.tile([128, 1], f32, name="r", tag=f"r{h}")
                        nc.vector.reciprocal(out=r, in_=s)
                        nc.gpsimd.dma_start(
                            out=rrows[(qb, h)][0:1,
                                               (ms % 4) * 128:(ms % 4 + 1) * 128],
                            in_=r[:, 0:1],
                        )
                        for uu, kh in ((u0, 0), (u, 1)):
                            a = apool.tile([128, 1024], f32, name="a", tag="a")
                            nc.vector.tensor_scalar_mul(a, uu, r[:, 0:1])
                            nc.sync.dma_start(
                                out=attn_out[h, ms * 128:(ms + 1) * 128,
                                             kh * 1024:(kh + 1) * 1024],
                                in_=a,
                            )
                # ---- two T mini-strips, heads packed --------------------
                q0 = qb * QB
                for ks in (2 * un, 2 * un + 1):
                    if ks == 0:
                        po_tiles[qb] = pp_o.tile(
                            [128, QB], f32, name="po", tag="po")
                    po = po_tiles[qb]
                    pt_t = [pp_t.tile([128, QB], f32, name="pt", tag="pt")
                            for _ in range(H_LOC)]
                    for h in range(H_LOC):
                        nc.tensor.matmul(
                            pt_t[h],
                            lhsT=khT_z[h][:, ks * 128:(ks + 1) * 128],
                            rhs=qhT_z[h][:, q0:q0 + QB],
                            start=True,
                            stop=True,
                        )
                    uts = []
                    for h in range(H_LOC):
                        ut = utpool.tile([128, QB], f16, name="ut", tag="ut")
                        nc.scalar.activation(
                            out=ut, in_=pt_t[h], func=AF.Exp, scale=float(SCALE))
                        uts.append(ut)
                    for h in range(H_LOC):
                        nc.tensor.matmul(
                            po[h * D_K:(h + 1) * D_K, :],
                            lhsT=vhb[:, ks, h * D_K:(h + 1) * D_K],
                            rhs=uts[h],
                            start=(ks == 0),
                            stop=(ks == NSTRIP - 1),
                            skip_group_check=True,
                        )
                    if ks == NSTRIP - 1:
                        pending.append(
                            (tick + 6,
                             (qb, po, rrows[(qb, 0)], rrows[(qb, 1)])))
        while pending:
            finalize_o(*pending.pop(0)[1])

        # ---- output projection: full K=128 ------------------------------
        for bc in range(NSTRIP):
            pout = pp_n.tile([128, 1024], f32, name="pout", tag="pn")
            for oc in range(2):
                nc.tensor.matmul(
                    pout[:, oc * 512:(oc + 1) * 512],
                    lhsT=o_sb[:, bc * 128:(bc + 1) * 128],
                    rhs=wo_sb[:, oc * 512:(oc + 1) * 512],
                    start=True,
                    stop=True,
                )
            osb = outpool.tile([128, 1024], f32, name="osb", tag="osb")
            nc.vector.tensor_copy(out=osb, in_=pout)
            nc.sync.dma_start(out=outp[bc * 128:(bc + 1) * 128, :], in_=osb)

    nc.compile()
    return nc


def _get_nc():
    if "nc" not in _CACHE:
        _CACHE["nc"] = _build_bass()
    return _CACHE["nc"]


def _make_in_maps(q, k, v, Wq, bq, Wk, bk, Wv, Wo):
    qT = np.ascontiguousarray(q.T.astype(np.float16))
    kT = np.ascontiguousarray(k.T.astype(np.float16))
    vT = np.ascontiguousarray(v.T.astype(np.float16))
    in_maps = []
    for c in range(N_CORES):
        sl = slice(c * HD, (c + 1) * HD)
        in_maps.append({
            "qT": qT,
            "kT": kT,
            "vT": vT,
            "wqT": np.ascontiguousarray(Wq[sl, :].T.astype(np.float16)),
            "wkT": np.ascontiguousarray(Wk[sl, :].T.astype(np.float16)),
            "wvT": np.ascontiguousarray(Wv[sl, :].T.astype(np.float16)),
            "woT": np.ascontiguousarray(Wo[:, c * HD:(c + 1) * HD].T.astype(np.float16)),
            "ones": np.ones((128, 128), dtype=np.float16),
            "bq": np.ascontiguousarray(bq[sl].reshape(HD, 1)),
            "bk": np.ascontiguousarray(bk[sl].reshape(HD, 1)),
        })
    return in_maps


def kernel(q, k, v, Wq, bq, Wk, bk, Wv, bv, Wo, bo):
    global LAST_RESULTS
    from concourse.bass_utils import run_bass_kernel_spmd

    q = np.ascontiguousarray(np.asarray(q, dtype=np.float32))
    k = np.ascontiguousarray(np.asarray(k, dtype=np.float32))
    v = np.ascontiguousarray(np.asarray(v, dtype=np.float32))
    Wq = np.asarray(Wq, dtype=np.float32)
    Wk = np.asarray(Wk, dtype=np.float32)
    Wv = np.asarray(Wv, dtype=np.float32)
    Wo = np.asarray(Wo, dtype=np.float32)
    bq = np.asarray(bq, dtype=np.float32)
    bk = np.asarray(bk, dtype=np.float32)
    bv = np.asarray(bv, dtype=np.float32)
    bo = np.asarray(bo, dtype=np.float32)

    in_maps = _make_in_maps(q, k, v, Wq, bq, Wk, bk, Wv, Wo)

    nc = _get_nc()
    res = run_bass_kernel_spmd(
        nc, in_maps, core_ids=list(range(N_CORES)),
    )
    LAST_RESULTS = res

    attn = np.concatenate([res.results[c]["attn"] for c in range(N_CORES)], axis=0)
    out = np.zeros((BS, D_OUT), dtype=np.float64)
    for c in range(N_CORES):
        out += res.results[c]["outp"]
    # bv folds through softmax (rows sum to 1) into a constant: Wo @ bv + bo
    out += (Wo.astype(np.float64) @ bv.astype(np.float64)) + bo.astype(np.float64)
    return out.astype(np.float32), attn


# revision 17
# speedup vs baseline: 2.2392x; 1.0067x over previous
"""Trainium2 Bass kernel for multi-head attention (BS=2048, D=1024, H=16, d_k=64).

Returns (output [2048,1024], attn [16,2048,2048]) like the reference.

Sharding: tensor-parallel over heads -- each of the 8 cores owns 2 heads.
Each core reads the full (host-pretransposed) q/k/v plus its head-slices of
the weights, computes its 2 heads' attention + attn output, writes its slice
of `attn` and a partial output projection.  Host sums the 8 partials and adds
the bias constants (bo + Wo@bv, which factor out exactly).

Per-core dataflow (all matmuls in float32r = full PE speed):
  - qhT/khT [128(head dims),2048] = W @ x^T projections (PSUM accum over 8
    k-chunks of D_IN, bias added on ScalarE eviction).
  - vh in natural [k-row, d] layout (lhsT = vT chunks), with a ones column
    appended so attn@V also produces softmax row sums.
  - phase N (per head, 16 q-strips): S = qhT^T @ khT -> PSUM [128,2048];
    ScalarE exp(0.125*S) with fused accum_out row-sums; VectorE reciprocal +
    tensor_scalar (per-partition) normalize; DMA the finished attn strip out.
  - phase T (per head, 2 q-halves, 16 k-strips): S^T = khT^T @ qhT; exp;
    attn@V accumulates O' [65,1024] over k-strips (row 64 = row sums);
    broadcast 1/s via a K=1 ones matmul, normalize O' on eviction.
  - output projection: out_part = O^T(both heads) @ WoT, accumulated as two
    K=64 matmuls per tile, evicted + DMA'd.
"""

import os
import sys
from contextlib import ExitStack

if "/opt/trn_rl_repo" not in sys.path:
    sys.path.insert(0, "/opt/trn_rl_repo")

import numpy as np

BS = 2048
D_IN = 1024
D_OUT = 1024
H = 16
D_K = 64
N_CORES = 8
H_LOC = H // N_CORES          # 2 heads per core
HD = H_LOC * D_K              # 128 head dims per core
KCH = D_IN // 128             # 8 contraction chunks for projections
NSTRIP = BS // 128            # 16 strips of 128
SCALE = 1.0 / np.sqrt(D_K)    # 0.125

_CACHE = {}

# Filled by the last run (for test.py): bass_utils.BassKernelResults
LAST_RESULTS = None


def _build_bass():
    import concourse.bass as bass
    import concourse.tile as tile
    import concourse.mybir as mybir
    from concourse import bacc

    f32 = mybir.dt.float32
    f16 = mybir.dt.float16
    AF = mybir.ActivationFunctionType

    nc = bacc.Bacc(None, target_bir_lowering=False)

    qT = nc.dram_tensor("qT", [D_IN, BS], f16, kind="ExternalInput")
    kT = nc.dram_tensor("kT", [D_IN, BS], f16, kind="ExternalInput")
    vT = nc.dram_tensor("vT", [D_IN, BS], f16, kind="ExternalInput")
    wqT = nc.dram_tensor("wqT", [D_IN, HD], f16, kind="ExternalInput")
    wkT = nc.dram_tensor("wkT", [D_IN, HD], f16, kind="ExternalInput")
    wvT = nc.dram_tensor("wvT", [D_IN, HD], f16, kind="ExternalInput")
    woT = nc.dram_tensor("woT", [HD, D_OUT], f16, kind="ExternalInput")
    onesd = nc.dram_tensor("ones", [128, 128], f16, kind="ExternalInput")
    bq = nc.dram_tensor("bq", [HD, 1], f32, kind="ExternalInput")
    bk = nc.dram_tensor("bk", [HD, 1], f32, kind="ExternalInput")

    attn_out = nc.dram_tensor("attn", [H_LOC, BS, BS], f32, kind="ExternalOutput")
    outp = nc.dram_tensor("outp", [BS, D_OUT], f32, kind="ExternalOutput")

    QB = 512                     # q-quarter width for the T stream
    NQB = BS // QB               # 4 quarters

    with tile.TileContext(nc) as tc, ExitStack() as ctx:
        consts = ctx.enter_context(tc.tile_pool(name="consts", bufs=1))
        slabs = ctx.enter_context(tc.tile_pool(name="slabs", bufs=3))
        upool = ctx.enter_context(tc.tile_pool(name="u", bufs=6))
        utpool = ctx.enter_context(tc.tile_pool(name="ut", bufs=4))
        apool = ctx.enter_context(tc.tile_pool(name="a", bufs=4))
        spool = ctx.enter_context(tc.tile_pool(name="s", bufs=8))
        outpool = ctx.enter_context(tc.tile_pool(name="outsb", bufs=3))
        orawpool = ctx.enter_context(tc.tile_pool(name="oraw", bufs=3))
        rrowpool = ctx.enter_context(tc.tile_pool(name="rrow", bufs=6))

        # 8-bank PSUM budget:
        #   pn: 2 x [128,1024]f32 (4 banks) -- N scores (one slot per head),
        #       also vh-psum [128,8,128] and outproj [128,1024]
        #   pt: 2 x [128,512]f32 (2 banks)  -- T scores / proj accum / B bcast
        #   po: 2 x [128,512]f32 (2 banks)  -- attn@V accum, both heads packed
        pp_n = ctx.enter_context(tc.tile_pool(name="pp_n", bufs=2, space="PSUM"))
        pp_t = ctx.enter_context(tc.tile_pool(name="pp_t", bufs=1, space="PSUM"))
        pp_o = ctx.enter_context(tc.tile_pool(name="pp_o", bufs=2, space="PSUM"))

        # ---- constants -------------------------------------------------
        w_q = consts.tile([128, KCH, HD], f16, name="w_q", tag="w_q")
        w_k = consts.tile([128, KCH, HD], f16, name="w_k", tag="w_k")
        w_v = consts.tile([128, KCH, HD], f16, name="w_v", tag="w_v")
        nc.sync.dma_start(out=w_q, in_=wqT.rearrange("(ko p) m -> p ko m", p=128))
        nc.sync.dma_start(out=w_k, in_=wkT.rearrange("(ko p) m -> p ko m", p=128))
        nc.sync.dma_start(out=w_v, in_=wvT.rearrange("(ko p) m -> p ko m", p=128))
        wo_sb = consts.tile([HD, D_OUT], f16, name="wo_sb", tag="wo_sb")
        nc.sync.dma_start(out=wo_sb, in_=woT[:, :])
        bq_sb = consts.tile([HD, 1], f32, name="bq_sb", tag="bq_sb")
        bk_sb = consts.tile([HD, 1], f32, name="bk_sb", tag="bk_sb")
        nc.sync.dma_start(out=bq_sb, in_=bq[:, :])
        nc.sync.dma_start(out=bk_sb, in_=bk[:, :])
        ones_sb = consts.tile([128, 128], f16, name="ones_sb", tag="ones_sb")
        nc.sync.dma_start(out=ones_sb, in_=onesd[:, :])

        # per-head K-padded projections: other head's partitions are zero so
        # every score matmul runs with a full K=128 contraction (full-array
        # activity keeps the PE clock unthrottled; zeros contribute nothing)
        qhT_z = [consts.tile([128, BS], f16, name=f"qhT_z{h}", tag=f"qhT_z{h}")
                 for h in range(H_LOC)]
        khT_z = [consts.tile([128, BS], f16, name=f"khT_z{h}", tag=f"khT_z{h}")
                 for h in range(H_LOC)]
        nc.vector.memset(qhT_z[0][D_K:128, :], 0.0)
        nc.vector.memset(qhT_z[1][0:D_K, :], 0.0)
        nc.vector.memset(khT_z[0][D_K:128, :], 0.0)
        nc.vector.memset(khT_z[1][0:D_K, :], 0.0)
        # natural-layout v heads, both packed: cols 0-63 h0, 64-127 h1
        vhb = consts.tile([128, NSTRIP, HD], f16, name="vhb", tag="vhb")
        o_sb = consts.tile([HD, BS], f16, name="o_sb", tag="o_sb")

        # ---- vh first (vT stream) --------------------------------------
        psv = [pp_n.tile([128, 8, 128], f32, name=f"psv{x}", tag="pn")
               for x in range(2)]
        for nb in range(4):
            vs = slabs.tile([128, KCH, QB], f16, name="vslab", tag="slab")
            nc.sync.dma_start(
                out=vs,
                in_=vT[:, nb * QB:(nb + 1) * QB].rearrange(
                    "(ko p) n -> p ko n", p=128),
            )
            for kc in range(KCH):
                for bx in range(4):
                    bc = nb * 4 + bx
                    nc.tensor.matmul(
                        psv[bc // 8][:, bc % 8, :],
                        lhsT=vs[:, kc, bx * 128:(bx + 1) * 128],
                        rhs=w_v[:, kc, :],
                        start=(kc == 0 and bc % 4 == 0),
                        stop=(kc == KCH - 1),
                        skip_group_check=True,
                    )
            for bx in range(4):
                bc = nb * 4 + bx
                with nc.allow_low_precision("fp16 v-heads feed fp16 matmul"):
                    nc.vector.tensor_copy(
                        out=vhb[:, bc, :],
                        in_=psv[bc // 8][:, bc % 8, :],
                    )

        # ---- k/q projections, q-block-major ----------------------------
        def project(x_dram, w_sb, b_sb, dst):
            for nb in range(4):
                slab = slabs.tile([128, KCH, QB], f16, name="slab", tag="slab")
                nc.sync.dma_start(
                    out=slab,
                    in_=x_dram[:, nb * QB:(nb + 1) * QB].rearrange(
                        "(ko p) n -> p ko n", p=128),
                )
                ps = pp_t.tile([128, QB], f32, name="pj", tag="pt")
                for kc in range(KCH):
                    nc.tensor.matmul(
                        ps,
                        lhsT=w_sb[:, kc, :],
                        rhs=slab[:, kc, :],
                        start=(kc == 0),
                        stop=(kc == KCH - 1),
                    )
                with nc.allow_low_precision("fp16 projections feed fp16 matmul"):
                    nc.vector.tensor_scalar_add(
                        dst[0][0:D_K, nb * QB:(nb + 1) * QB],
                        ps[0:D_K, :], b_sb[0:D_K, 0:1],
                    )
                    nc.vector.tensor_scalar_add(
                        dst[1][D_K:128, nb * QB:(nb + 1) * QB],
                        ps[D_K:128, :], b_sb[D_K:128, 0:1],
                    )

        project(kT, w_k, bk_sb, khT_z)
        project(qT, w_q, bq_sb, qhT_z)

        # ---- attention: 32 units, both heads interleaved ----------------
        def finalize_o(qb, po, rrow0, rrow1):
            """Copy O' (both heads) out of PSUM, normalize by per-head 1/s
            rows broadcast over each head's partition range."""
            o_raw = orawpool.tile([128, QB], f32, name="o_raw", tag="o_raw")
            nc.vector.tensor_copy(out=o_raw, in_=po)
            pb = pp_t.tile([128, QB], f32, name="pb", tag="pt")
            nc.tensor.matmul(
                pb[0:D_K, :], lhsT=ones_sb[0:1, 0:D_K], rhs=rrow0[0:1, :],
                start=True, stop=True, skip_group_check=True,
            )
            nc.tensor.matmul(
                pb[D_K:HD, :], lhsT=ones_sb[0:1, 0:D_K], rhs=rrow1[0:1, :],
                start=True, stop=True, skip_group_check=True,
            )
            with nc.allow_low_precision("O output feeds fp16 out-projection"):
                nc.vector.tensor_mul(
                    o_sb[:, qb * QB:(qb + 1) * QB], o_raw, pb,
                )

        pending = []
        tick = 0
        po_tiles = {}
        rrows = {}
        hold = {}
        for qb in range(NQB):
            for un in range(8):
                while pending and pending[0][0] <= tick:
                    finalize_o(*pending.pop(0)[1])
                tick += 1
                ms, khalf = qb * 4 + un // 2, un % 2
                k0 = khalf * 1024
                if un == 0:
                    for h in range(H_LOC):
                        rrows[(qb, h)] = rrowpool.tile(
                            [1, QB], f16, name="rrow", tag="rrow")
                # ---- N half-strips, both heads, row-group interleaved ---
                pn_t = [pp_n.tile([128, 1024], f32, name="pn", tag="pn")
                        for _ in range(H_LOC)]
                for nb in range(2):
                    for h in range(H_LOC):
                        nc.tensor.matmul(
                            pn_t[h][:, nb * 512:(nb + 1) * 512],
                            lhsT=qhT_z[h][:, ms * 128:(ms + 1) * 128],
                            rhs=khT_z[h][:, k0 + nb * 512:k0 + (nb + 1) * 512],
                            start=True,
                            stop=True,
                        )
                for h in range(H_LOC):
                    u = upool.tile([128, 1024], f32, name="u", tag="u")
                    sp = spool.tile([128, 1], f32, name="sp", tag=f"sp{khalf}{h}")
                    nc.scalar.activation(
                        out=u, in_=pn_t[h], func=AF.Exp, scale=float(SCALE),
                        accum_out=sp,
                    )
                    if khalf == 0:
                        hold[h] = (u, sp)
                    else:
                        u0, s0 = hold[h]
                        s = spool.tile([128, 1], f32, name="s", tag=f"s{h}")
                        nc.vector.tensor_add(s, s0, sp)
                        r = spool.tile([128, 1], f32, name="r", tag=f"r{h}")
                        nc.vector.reciprocal(out=r, in_=s)
                        nc.gpsimd.dma_start(
                            out=rrows[(qb, h)][0:1,
                                               (ms % 4) * 128:(ms % 4 + 1) * 128],
                            in_=r[:, 0:1],
                        )
                        for uu, kh in ((u0, 0), (u, 1)):
                            a = apool.tile([128, 1024], f32, name="a", tag="a")
                            nc.vector.tensor_scalar_mul(a, uu, r[:, 0:1])
                            nc.sync.dma_start(
                                out=attn_out[h, ms * 128:(ms + 1) * 128,
                                             kh * 1024:(kh + 1) * 1024],
                                in_=a,
                            )
                # ---- two T mini-strips, heads packed --------------------
                q0 = qb * QB
                for ks in (2 * un, 2 * un + 1):
                    if ks == 0:
                        po_tiles[qb] = pp_o.tile(
                            [128, QB], f32, name="po", tag="po")
                    po = po_tiles[qb]
                    pt = pp_t.tile([128, H_LOC, QB], f32, name="pt", tag="pt")
                    for h in range(H_LOC):
                        nc.tensor.matmul(
                            pt[:, h, :],
                            lhsT=khT_z[h][:, ks * 128:(ks + 1) * 128],
                            rhs=qhT_z[h][:, q0:q0 + QB],
                            start=True,
                            stop=True,
                        )
                    ut = utpool.tile([128, H_LOC * QB], f16, name="ut", tag="ut")
                    nc.scalar.activation(
                        out=ut, in_=pt, func=AF.Exp, scale=float(SCALE))
                    for h in range(H_LOC):
                        nc.tensor.matmul(
                            po[h * D_K:(h + 1) * D_K, :],
                            lhsT=vhb[:, ks, h * D_K:(h + 1) * D_K],
                            rhs=ut[:, h * QB:(h + 1) * QB],
                            start=(ks == 0),
                            stop=(ks == NSTRIP - 1),
                            skip_group_check=True,
                        )
                    if ks == NSTRIP - 1:
                        pending.append(
                            (tick + 6,
                             (qb, po, rrows[(qb, 0)], rrows[(qb, 1)])))
        while pending:
            finalize_o(*pending.pop(0)[1])

        # ---- output projection: full K=128 ------------------------------
        for bc in range(NSTRIP):
            pout = pp_n.tile([128, 1024], f32, name="pout", tag="pn")
            for oc in range(2):
                nc.tensor.matmul(
                    pout[:, oc * 512:(oc + 1) * 512],
                    lhsT=o_sb[:, bc * 128:(bc + 1) * 128],
                    rhs=wo_sb[:, oc * 512:(oc + 1) * 512],
                    start=True,
                    stop=True,
                )
            osb = outpool.tile([128, 1024], f32, name="osb", tag="osb")
            nc.vector.tensor_copy(out=osb, in_=pout)
            nc.sync.dma_start(out=outp[bc * 128:(bc + 1) * 128, :], in_=osb)

    nc.compile()
    return nc


def _get_nc():
    if "nc" not in _CACHE:
        _CACHE["nc"] = _build_bass()
    return _CACHE["nc"]


def _make_in_maps(q, k, v, Wq, bq, Wk, bk, Wv, Wo):
    qT = np.ascontiguousarray(q.T.astype(np.float16))
    kT = np.ascontiguousarray(k.T.astype(np.float16))
    vT = np.ascontiguousarray(v.T.astype(np.float16))
    in_maps = []
    for c in range(N_CORES):
        sl = slice(c * HD, (c + 1) * HD)
        in_maps.append({
            "qT": qT,
            "kT": kT,
            "vT": vT,
            "wqT": np.ascontiguousarray(Wq[sl, :].T.astype(np.float16)),
            "wkT": np.ascontiguousarray(Wk[sl, :].T.astype(np.float16)),
            "wvT": np.ascontiguousarray(Wv[sl, :].T.astype(np.float16)),
            "woT": np.ascontiguousarray(Wo[:, c * HD:(c + 1) * HD].T.astype(np.float16)),
            "ones": np.ones((128, 128), dtype=np.float16),
            "bq": np.ascontiguousarray(bq[sl].reshape(HD, 1)),
            "bk": np.ascontiguousarray(bk[sl].reshape(HD, 1)),
        })
    return in_maps


def kernel(q, k, v, Wq, bq, Wk, bk, Wv, bv, Wo, bo):
    global LAST_RESULTS
    from concourse.bass_utils import run_bass_kernel_spmd

    q = np.ascontiguousarray(np.asarray(q, dtype=np.float32))
    k = np.ascontiguousarray(np.asarray(k, dtype=np.float32))
    v = np.ascontiguousarray(np.asarray(v, dtype=np.float32))
    Wq = np.asarray(Wq, dtype=np.float32)
    Wk = np.asarray(Wk, dtype=np.float32)
    Wv = np.asarray(Wv, dtype=np.float32)
    Wo = np.asarray(Wo, dtype=np.float32)
    bq = np.asarray(bq, dtype=np.float32)
    bk = np.asarray(bk, dtype=np.float32)
    bv = np.asarray(bv, dtype=np.float32)
    bo = np.asarray(bo, dtype=np.float32)

    in_maps = _make_in_maps(q, k, v, Wq, bq, Wk, bk, Wv, Wo)

    nc = _get_nc()
    res = run_bass_kernel_spmd(
        nc, in_maps, core_ids=list(range(N_CORES)),
    )
    LAST_RESULTS = res

    attn = np.concatenate([res.results[c]["attn"] for c in range(N_CORES)], axis=0)
    out = np.zeros((BS, D_OUT), dtype=np.float64)
    for c in range(N_CORES):
        out += res.results[c]["outp"]
    # bv folds through softmax (rows sum to 1) into a constant: Wo @ bv + bo
    out += (Wo.astype(np.float64) @ bv.astype(np.float64)) + bo.astype(np.float64)
    return out.astype(np.float32), attn


# revision 18
# speedup vs baseline: 2.3079x; 1.0307x over previous
"""Trainium2 Bass kernel for multi-head attention (BS=2048, D=1024, H=16, d_k=64).

Returns (output [2048,1024], attn [16,2048,2048]) like the reference.

Sharding: tensor-parallel over heads -- each of the 8 cores owns 2 heads.
Each core reads the full (host-pretransposed) q/k/v plus its head-slices of
the weights, computes its 2 heads' attention + attn output, writes its slice
of `attn` and a partial output projection.  Host sums the 8 partials and adds
the bias constants (bo + Wo@bv, which factor out exactly).

Per-core dataflow (all matmuls in float32r = full PE speed):
  - qhT/khT [128(head dims),2048] = W @ x^T projections (PSUM accum over 8
    k-chunks of D_IN, bias added on ScalarE eviction).
  - vh in natural [k-row, d] layout (lhsT = vT chunks), with a ones column
    appended so attn@V also produces softmax row sums.
  - phase N (per head, 16 q-strips): S = qhT^T @ khT -> PSUM [128,2048];
    ScalarE exp(0.125*S) with fused accum_out row-sums; VectorE reciprocal +
    tensor_scalar (per-partition) normalize; DMA the finished attn strip out.
  - phase T (per head, 2 q-halves, 16 k-strips): S^T = khT^T @ qhT; exp;
    attn@V accumulates O' [65,1024] over k-strips (row 64 = row sums);
    broadcast 1/s via a K=1 ones matmul, normalize O' on eviction.
  - output projection: out_part = O^T(both heads) @ WoT, accumulated as two
    K=64 matmuls per tile, evicted + DMA'd.
"""

import os
import sys
from contextlib import ExitStack

if "/opt/trn_rl_repo" not in sys.path:
    sys.path.insert(0, "/opt/trn_rl_repo")

import numpy as np

BS = 2048
D_IN = 1024
D_OUT = 1024
H = 16
D_K = 64
N_CORES = 8
H_LOC = H // N_CORES          # 2 heads per core
HD = H_LOC * D_K              # 128 head dims per core
KCH = D_IN // 128             # 8 contraction chunks for projections
NSTRIP = BS // 128            # 16 strips of 128
SCALE = 1.0 / np.sqrt(D_K)    # 0.125

_CACHE = {}

# Filled by the last run (for test.py): bass_utils.BassKernelResults
LAST_RESULTS = None


def _build_bass():
    import concourse.bass as bass
    import concourse.tile as tile
    import concourse.mybir as mybir
    from concourse import bacc

    f32 = mybir.dt.float32
    f16 = mybir.dt.float16
    AF = mybir.ActivationFunctionType

    nc = bacc.Bacc(None, target_bir_lowering=False)

    qT = nc.dram_tensor("qT", [4, 128, KCH, 512], f16, kind="ExternalInput")
    kT = nc.dram_tensor("kT", [4, 128, KCH, 512], f16, kind="ExternalInput")
    vT = nc.dram_tensor("vT", [4, 128, KCH, 512], f16, kind="ExternalInput")
    wqT = nc.dram_tensor("wqT", [D_IN, HD], f16, kind="ExternalInput")
    wkT = nc.dram_tensor("wkT", [D_IN, HD], f16, kind="ExternalInput")
    wvT = nc.dram_tensor("wvT", [D_IN, HD], f16, kind="ExternalInput")
    woT = nc.dram_tensor("woT", [HD, D_OUT], f16, kind="ExternalInput")
    onesd = nc.dram_tensor("ones", [128, 128], f16, kind="ExternalInput")
    bq = nc.dram_tensor("bq", [HD, 1], f32, kind="ExternalInput")
    bk = nc.dram_tensor("bk", [HD, 1], f32, kind="ExternalInput")

    attn_out = nc.dram_tensor("attn", [H_LOC, BS, BS], f32, kind="ExternalOutput")
    outp = nc.dram_tensor("outp", [BS, D_OUT], f32, kind="ExternalOutput")

    QB = 512                     # q-quarter width for the T stream
    NQB = BS // QB               # 4 quarters

    with tile.TileContext(nc) as tc, ExitStack() as ctx:
        consts = ctx.enter_context(tc.tile_pool(name="consts", bufs=1))
        slabs = ctx.enter_context(tc.tile_pool(name="slabs", bufs=3))
        upool = ctx.enter_context(tc.tile_pool(name="u", bufs=6))
        utpool = ctx.enter_context(tc.tile_pool(name="ut", bufs=4))
        apool = ctx.enter_context(tc.tile_pool(name="a", bufs=4))
        spool = ctx.enter_context(tc.tile_pool(name="s", bufs=8))
        outpool = ctx.enter_context(tc.tile_pool(name="outsb", bufs=3))
        orawpool = ctx.enter_context(tc.tile_pool(name="oraw", bufs=3))
        rrowpool = ctx.enter_context(tc.tile_pool(name="rrow", bufs=6))

        # 8-bank PSUM budget:
        #   pn: 2 x [128,1024]f32 (4 banks) -- N scores (one slot per head),
        #       also vh-psum [128,8,128] and outproj [128,1024]
        #   pt: 2 x [128,512]f32 (2 banks)  -- T scores / proj accum / B bcast
        #   po: 2 x [128,512]f32 (2 banks)  -- attn@V accum, both heads packed
        pp_n = ctx.enter_context(tc.tile_pool(name="pp_n", bufs=2, space="PSUM"))
        pp_t = ctx.enter_context(tc.tile_pool(name="pp_t", bufs=1, space="PSUM"))
        pp_o = ctx.enter_context(tc.tile_pool(name="pp_o", bufs=2, space="PSUM"))

        # ---- constants -------------------------------------------------
        w_q = consts.tile([128, KCH, HD], f16, name="w_q", tag="w_q")
        w_k = consts.tile([128, KCH, HD], f16, name="w_k", tag="w_k")
        w_v = consts.tile([128, KCH, HD], f16, name="w_v", tag="w_v")
        nc.sync.dma_start(out=w_q, in_=wqT.rearrange("(ko p) m -> p ko m", p=128))
        nc.sync.dma_start(out=w_k, in_=wkT.rearrange("(ko p) m -> p ko m", p=128))
        nc.sync.dma_start(out=w_v, in_=wvT.rearrange("(ko p) m -> p ko m", p=128))
        wo_sb = consts.tile([HD, D_OUT], f16, name="wo_sb", tag="wo_sb")
        nc.sync.dma_start(out=wo_sb, in_=woT[:, :])
        bq_sb = consts.tile([HD, 1], f32, name="bq_sb", tag="bq_sb")
        bk_sb = consts.tile([HD, 1], f32, name="bk_sb", tag="bk_sb")
        nc.sync.dma_start(out=bq_sb, in_=bq[:, :])
        nc.sync.dma_start(out=bk_sb, in_=bk[:, :])
        ones_sb = consts.tile([128, 128], f16, name="ones_sb", tag="ones_sb")
        nc.sync.dma_start(out=ones_sb, in_=onesd[:, :])

        # per-head K-padded projections: other head's partitions are zero so
        # every score matmul runs with a full K=128 contraction (full-array
        # activity keeps the PE clock unthrottled; zeros contribute nothing)
        qhT_z = [consts.tile([128, BS], f16, name=f"qhT_z{h}", tag=f"qhT_z{h}")
                 for h in range(H_LOC)]
        khT_z = [consts.tile([128, BS], f16, name=f"khT_z{h}", tag=f"khT_z{h}")
                 for h in range(H_LOC)]
        nc.vector.memset(qhT_z[0][D_K:128, :], 0.0)
        nc.vector.memset(qhT_z[1][0:D_K, :], 0.0)
        nc.vector.memset(khT_z[0][D_K:128, :], 0.0)
        nc.vector.memset(khT_z[1][0:D_K, :], 0.0)
        # natural-layout v heads, both packed: cols 0-63 h0, 64-127 h1
        vhb = consts.tile([128, NSTRIP, HD], f16, name="vhb", tag="vhb")
        o_sb = consts.tile([HD, BS], f16, name="o_sb", tag="o_sb")

        # ---- k/q projections, q-block-major ----------------------------
        def proj_block(x_dram, nb, w_sb, b_sb, dst):
                slab = slabs.tile([128, KCH, QB], f16, name="slab", tag="slab")
                nc.sync.dma_start(out=slab, in_=x_dram[nb])
                ps = pp_t.tile([128, QB], f32, name="pj", tag="pt")
                for kc in range(KCH):
                    nc.tensor.matmul(
                        ps,
                        lhsT=w_sb[:, kc, :],
                        rhs=slab[:, kc, :],
                        start=(kc == 0),
                        stop=(kc == KCH - 1),
                    )
                with nc.allow_low_precision("fp16 projections feed fp16 matmul"):
                    nc.vector.tensor_scalar_add(
                        dst[0][0:D_K, nb * QB:(nb + 1) * QB],
                        ps[0:D_K, :], b_sb[0:D_K, 0:1],
                    )
                    nc.vector.tensor_scalar_add(
                        dst[1][D_K:128, nb * QB:(nb + 1) * QB],
                        ps[D_K:128, :], b_sb[D_K:128, 0:1],
                    )

        for nb in range(4):
            proj_block(kT, nb, w_k, bk_sb, khT_z)
        proj_block(qT, 0, w_q, bq_sb, qhT_z)

        # ---- vh (vT stream, after k/q head-start blocks) ---------------
        psv = [pp_n.tile([128, 8, 128], f32, name=f"psv{x}", tag="pn")
               for x in range(2)]
        for nb in range(4):
            vs = slabs.tile([128, KCH, QB], f16, name="vslab", tag="slab")
            nc.sync.dma_start(out=vs, in_=vT[nb])
            for kc in range(KCH):
                for bx in range(4):
                    bc = nb * 4 + bx
                    nc.tensor.matmul(
                        psv[bc // 8][:, bc % 8, :],
                        lhsT=vs[:, kc, bx * 128:(bx + 1) * 128],
                        rhs=w_v[:, kc, :],
                        start=(kc == 0 and bc % 4 == 0),
                        stop=(kc == KCH - 1),
                        skip_group_check=True,
                    )
            for bx in range(4):
                bc = nb * 4 + bx
                with nc.allow_low_precision("fp16 v-heads feed fp16 matmul"):
                    nc.vector.tensor_copy(
                        out=vhb[:, bc, :],
                        in_=psv[bc // 8][:, bc % 8, :],
                    )

        for nb in range(1, 4):
            proj_block(qT, nb, w_q, bq_sb, qhT_z)

        # ---- attention: 32 units, both heads interleaved ----------------
        def finalize_o(qb, po, rrow0, rrow1):
            """Copy O' (both heads) out of PSUM, normalize by per-head 1/s
            rows broadcast over each head's partition range."""
            o_raw = orawpool.tile([128, QB], f32, name="o_raw", tag="o_raw")
            nc.vector.tensor_copy(out=o_raw, in_=po)
            pb = pp_t.tile([128, QB], f32, name="pb", tag="pt")
            nc.tensor.matmul(
                pb[0:D_K, :], lhsT=ones_sb[0:1, 0:D_K], rhs=rrow0[0:1, :],
                start=True, stop=True, skip_group_check=True,
            )
            nc.tensor.matmul(
                pb[D_K:HD, :], lhsT=ones_sb[0:1, 0:D_K], rhs=rrow1[0:1, :],
                start=True, stop=True, skip_group_check=True,
            )
            with nc.allow_low_precision("O output feeds fp16 out-projection"):
                nc.vector.tensor_mul(
                    o_sb[:, qb * QB:(qb + 1) * QB], o_raw, pb,
                )

        pending = []
        tick = 0
        po_tiles = {}
        rrows = {}
        hold = {}
        for qb in range(NQB):
            for un in range(8):
                while pending and pending[0][0] <= tick:
                    finalize_o(*pending.pop(0)[1])
                tick += 1
                ms, khalf = qb * 4 + un // 2, un % 2
                k0 = khalf * 1024
                if un == 0:
                    for h in range(H_LOC):
                        rrows[(qb, h)] = rrowpool.tile(
                            [1, QB], f16, name="rrow", tag="rrow")
                # ---- N half-strips, both heads, row-group interleaved ---
                pn_t = [pp_n.tile([128, 1024], f32, name="pn", tag="pn")
                        for _ in range(H_LOC)]
                for nb in range(2):
                    for h in range(H_LOC):
                        nc.tensor.matmul(
                            pn_t[h][:, nb * 512:(nb + 1) * 512],
                            lhsT=qhT_z[h][:, ms * 128:(ms + 1) * 128],
                            rhs=khT_z[h][:, k0 + nb * 512:k0 + (nb + 1) * 512],
                            start=True,
                            stop=True,
                        )
                for h in range(H_LOC):
                    u = upool.tile([128, 1024], f32, name="u", tag="u")
                    sp = spool.tile([128, 1], f32, name="sp", tag=f"sp{khalf}{h}")
                    nc.scalar.activation(
                        out=u, in_=pn_t[h], func=AF.Exp, scale=float(SCALE),
                        accum_out=sp,
                    )
                    if khalf == 0:
                        hold[h] = (u, sp)
                    else:
                        u0, s0 = hold[h]
                        s = spool.tile([128, 1], f32, name="s", tag=f"s{h}")
                        nc.vector.tensor_add(s, s0, sp)
                        r = spool.tile([128, 1], f32, name="r", tag=f"r{h}")
                        nc.vector.reciprocal(out=r, in_=s)
                        nc.gpsimd.dma_start(
                            out=rrows[(qb, h)][0:1,
                                               (ms % 4) * 128:(ms % 4 + 1) * 128],
                            in_=r[:, 0:1],
                        )
                        for uu, kh in ((u0, 0), (u, 1)):
                            a = apool.tile([128, 1024], f32, name="a", tag="a")
                            nc.vector.tensor_scalar_mul(a, uu, r[:, 0:1])
                            nc.sync.dma_start(
                                out=attn_out[h, ms * 128:(ms + 1) * 128,
                                             kh * 1024:(kh + 1) * 1024],
                                in_=a,
                            )
                # ---- two T mini-strips, heads packed --------------------
                q0 = qb * QB
                for ks in (2 * un, 2 * un + 1):
                    if ks == 0:
                        po_tiles[qb] = pp_o.tile(
                            [128, QB], f32, name="po", tag="po")
                    po = po_tiles[qb]
                    pt = pp_t.tile([128, H_LOC, QB], f32, name="pt", tag="pt")
                    for h in range(H_LOC):
                        nc.tensor.matmul(
                            pt[:, h, :],
                            lhsT=khT_z[h][:, ks * 128:(ks + 1) * 128],
                            rhs=qhT_z[h][:, q0:q0 + QB],
                            start=True,
                            stop=True,
                        )
                    ut = utpool.tile([128, H_LOC * QB], f16, name="ut", tag="ut")
                    nc.scalar.activation(
                        out=ut, in_=pt, func=AF.Exp, scale=float(SCALE))
                    for h in range(H_LOC):
                        nc.tensor.matmul(
                            po[h * D_K:(h + 1) * D_K, :],
                            lhsT=vhb[:, ks, h * D_K:(h + 1) * D_K],
                            rhs=ut[:, h * QB:(h + 1) * QB],
                            start=(ks == 0),
                            stop=(ks == NSTRIP - 1),
                            skip_group_check=True,
                        )
                    if ks == NSTRIP - 1:
                        pending.append(
                            (tick + 6,
                             (qb, po, rrows[(qb, 0)], rrows[(qb, 1)])))
        while pending:
            finalize_o(*pending.pop(0)[1])

        # ---- output projection: full K=128 ------------------------------
        for bc in range(NSTRIP):
            pout = pp_n.tile([128, 1024], f32, name="pout", tag="pn")
            for oc in range(2):
                nc.tensor.matmul(
                    pout[:, oc * 512:(oc + 1) * 512],
                    lhsT=o_sb[:, bc * 128:(bc + 1) * 128],
                    rhs=wo_sb[:, oc * 512:(oc + 1) * 512],
                    start=True,
                    stop=True,
                )
            osb = outpool.tile([128, 1024], f32, name="osb", tag="osb")
            nc.vector.tensor_copy(out=osb, in_=pout)
            nc.sync.dma_start(out=outp[bc * 128:(bc + 1) * 128, :], in_=osb)

    nc.compile()
    return nc


def _get_nc():
    if "nc" not in _CACHE:
        _CACHE["nc"] = _build_bass()
    return _CACHE["nc"]


def _make_in_maps(q, k, v, Wq, bq, Wk, bk, Wv, Wo):
    def _blocked(x):
        # [BS, D_IN] -> [4, 128, 8, 512]: slab nb holds x^T chunk
        # [p, ko, n] = x[nb*512+n, ko*128+p]
        return np.ascontiguousarray(
            x.astype(np.float16).reshape(4, 512, KCH, 128).transpose(0, 3, 2, 1))

    qT = _blocked(q)
    kT = _blocked(k)
    vT = _blocked(v)
    in_maps = []
    for c in range(N_CORES):
        sl = slice(c * HD, (c + 1) * HD)
        in_maps.append({
            "qT": qT,
            "kT": kT,
            "vT": vT,
            "wqT": np.ascontiguousarray(Wq[sl, :].T.astype(np.float16)),
            "wkT": np.ascontiguousarray(Wk[sl, :].T.astype(np.float16)),
            "wvT": np.ascontiguousarray(Wv[sl, :].T.astype(np.float16)),
            "woT": np.ascontiguousarray(Wo[:, c * HD:(c + 1) * HD].T.astype(np.float16)),
            "ones": np.ones((128, 128), dtype=np.float16),
            "bq": np.ascontiguousarray(bq[sl].reshape(HD, 1)),
            "bk": np.ascontiguousarray(bk[sl].reshape(HD, 1)),
        })
    return in_maps


def kernel(q, k, v, Wq, bq, Wk, bk, Wv, bv, Wo, bo):
    global LAST_RESULTS
    from concourse.bass_utils import run_bass_kernel_spmd

    q = np.ascontiguousarray(np.asarray(q, dtype=np.float32))
    k = np.ascontiguousarray(np.asarray(k, dtype=np.float32))
    v = np.ascontiguousarray(np.asarray(v, dtype=np.float32))
    Wq = np.asarray(Wq, dtype=np.float32)
    Wk = np.asarray(Wk, dtype=np.float32)
    Wv = np.asarray(Wv, dtype=np.float32)
    Wo = np.asarray(Wo, dtype=np.float32)
    bq = np.asarray(bq, dtype=np.float32)
    bk = np.asarray(bk, dtype=np.float32)
    bv = np.asarray(bv, dtype=np.float32)
    bo = np.asarray(bo, dtype=np.float32)

    in_maps = _make_in_maps(q, k, v, Wq, bq, Wk, bk, Wv, Wo)

    nc = _get_nc()
    res = run_bass_kernel_spmd(
        nc, in_maps, core_ids=list(range(N_CORES)),
    )
    LAST_RESULTS = res

    attn = np.concatenate([res.results[c]["attn"] for c in range(N_CORES)], axis=0)
    out = np.zeros((BS, D_OUT), dtype=np.float64)
    for c in range(N_CORES):
        out += res.results[c]["outp"]
    # bv folds through softmax (rows sum to 1) into a constant: Wo @ bv + bo
    out += (Wo.astype(np.float64) @ bv.astype(np.float64)) + bo.astype(np.float64)
    return out.astype(np.float32), attn
